# revision 35
# baseline (speedup 1.0000x reference)
"""Trainium2 Bass kernel for MultiHeadFrequencyCrossAttention.

Math note: the reference computes, per (batch, head) slice,
    energy = ifft2( fft2(Q) @ fft2(K)^T * dk ).real
Because the DFT matrix F satisfies F @ F^T = n * P (P = index-negation
permutation), this collapses EXACTLY to
    energy = dk * D * Q @ K~^T        with K~[j, d] = K[j, (-d) mod D]
i.e. plain attention with K's head-dim index flipped (mod D) and an extra
scale of dk * D = 512.  No FFTs are needed; the flip and scale are folded
into host-side slices of the Wk / Wq projection weights.

Sharding: 8 cores = 4 batches x 2 head-groups (4 heads each).  Each core
gets q[b]^T, kv[b]^T (pre-transposed on host so the contraction dim lands
on SBUF partitions) plus its slice of the projection weights, computes
attention for its 4 heads, and applies its slice of Wo.  The host sums the
four partial Wo products per batch (two cores x two wo subgroups).

Precision scheme (PE fp32 matmuls are 4 cyc/row; fp16 is 1 cyc/row):
every value on the logit path is split hi/lo into two fp16 parts
(x = xh + xl, products of fp16 are exact in the fp32 PSUM accumulator), so
  x @ y ~= xh@yh + (xh@yl + xl@yh)     [~22-bit mantissa, err ~1e-6 rel]
One extra all-ones row in the stationary K operand times a "-rowmax" row
in the moving Q operand injects the softmax max-subtraction bias directly
into the S^T matmul.  The row max itself comes from a separate hi-only
fp16 pass (error ~ +-15 absolute on ~25000-scale logits, well inside the
exp() range window since A tiles are bf16).  A/V/output paths are plain
16-bit (error there stays relative, ~2e-3, no sharp-softmax blowup).

Engine assignment (v2 -- rebalanced so ACT only does exp/ln work and
gpsimd is not used at all, which avoids its library swaps):
  PE   : all matmuls (warmup, projections, max pass, S^T, AV, Wo)
  ACT  : A = exp(S^T - m) psum->sbuf, per-head ln(s) / exp(-ln s), and
         the proj hi evacuations (proj phase only, PE-bound there)
  DVE  : reduce_max (negated, fp16 out), proj lo subs, yun evacuation,
         y * (1/s) normalize (writes fp16 yh directly), wo psum->sbuf
  DMA  : inputs (chunked), colmax fp16 bounce straight into qm bias row,
         1/s row broadcast to 64 partitions (stride-0 descriptor),
         per-i-block output stores (separate out0/out1, host adds)

Emission order software-pipelines everything: maxpass(0)/(1) matmuls are
woven into the m=1 projection streams, maxpass(2)/(3) into mainpass(0)/(1)
j-blocks, wo(0) into mainpass(2), so PE never idles long enough for the
HAM clock gate to re-throttle (the v1 kernel spent 110us at half clock).
"""

import numpy as np
from contextlib import ExitStack

import concourse.bass as bass
import concourse.tile as tile
from concourse import bacc, mybir
from concourse.bass_utils import run_bass_kernel_spmd

F32 = mybir.dt.float32
F16 = mybir.dt.float16
BF16 = mybir.dt.bfloat16
AX = mybir.AxisListType
AF = mybir.ActivationFunctionType

T = 1024          # sequence length
E = 512           # embed dim
H = 8             # total heads
D = E // H        # head dim = 64
NH = 4            # heads per core
DX = NH * (D + 1) # vp columns incl. ones = 260
N_CORES = 8
SCALE = float(D) * float(D) ** 0.5  # dk * D = 512.0

TRACE = False          # set by test harness; adds NTFF profiling
LAST_EXEC_NS = None


def _emit(ctx, tc, dram):
    nc = tc.nc
    const = ctx.enter_context(tc.tile_pool(name="const", bufs=1))
    ps_big = ctx.enter_context(tc.tile_pool(name="ps_big", bufs=3, space="PSUM"))
    ps_av = ctx.enter_context(tc.tile_pool(name="ps_av", bufs=1, space="PSUM"))
    atp = ctx.enter_context(tc.tile_pool(name="atp", bufs=4))
    outp = ctx.enter_context(tc.tile_pool(name="outp", bufs=3))
    dramp = ctx.enter_context(tc.tile_pool(name="dramp", bufs=1, space="DRAM"))

    # ---- input loads (all fp16 on the wire) ----
    # The four big [E, T] tensors are split into T-halves so the first
    # projection block can start after ~1.5 MB instead of ~2.5 MB.
    tiles3 = {}
    def declare(name, cols):
        t3 = const.tile([128, 4, cols], F16, tag=name, name=name)
        tiles3[name] = t3
        return [t3[:, e, :] for e in range(4)]

    def load_half(name, half):
        t3 = tiles3[name]
        cols = t3.shape[2]
        h0, h1 = half * cols // 2, (half + 1) * cols // 2
        nc.sync.dma_start(
            t3[:, :, h0:h1],
            dram[name][:, h0:h1].rearrange("(c p) t -> p c t", p=128),
        )

    def load_full(name):
        t3 = tiles3[name]
        nc.sync.dma_start(
            t3[:], dram[name][:].rearrange("(c p) t -> p c t", p=128)
        )

    wqh = declare("wqh", NH * D)
    wql = declare("wql", NH * D)
    qh_in = declare("qh", T)
    ql_in = declare("ql", T)
    wkh = declare("wkh", NH * D)
    wkl = declare("wkl", NH * D)
    kvh_in = declare("kvh", T)
    kvl_in = declare("kvl", T)
    wv = declare("wv", DX)
    wo3 = const.tile([128, 2, E], F16, tag="wo", name="wo")
    wo = [wo3[:, g, :] for g in range(2)]
    ident = const.tile([128, 128], F32, tag="ident", name="ident")
    nc.sync.dma_start(ident[:], dram["ident"][:])

    # q-proj n=0 needs these four first
    load_full("wqh"); load_full("wql")
    load_half("ql", 0); load_half("qh", 0)
    load_half("ql", 1); load_half("qh", 1)
    load_full("wkh"); load_full("wkl")
    load_half("kvl", 0); load_half("kvh", 0)
    load_half("kvl", 1); load_half("kvh", 1)
    load_full("wv")
    nc.sync.dma_start(wo3[:], dram["wo"][:].rearrange("(g p) t -> p g t", p=128))

    # PE warm-up: dummy matmuls fill the input-DMA window so the HAM clock
    # gate is already at 8/8 (2.4 GHz) when the projections start.  18 MMs
    # x ~430ns cold spans the ~8us until the first projection operands land.
    wrm = const.tile([128, 512], F16, tag="wrm", name="wrm")
    nc.vector.memset(wrm[:], 0.0)
    ones_col = const.tile([1, 64], F32, tag="ones_col", name="ones_col")
    nc.vector.memset(ones_col[:], 1.0)
    # Dummy Exp/Ln so the natural_log_exp table set loads inside the input
    # DMA window instead of stalling ACT at mainpass(0)'s first exp.
    dtab = const.tile([1, 2], F32, tag="dtab", name="dtab")
    nc.scalar.activation(dtab[:], wrm[0:1, 0:2], AF.Exp)
    nc.scalar.activation(dtab[:], wrm[0:1, 0:2], AF.Ln)
    for w in range(18):
        pw = ps_big.tile([128, 512], F32, tag="big", name="psw")
        nc.tensor.matmul(pw[:], lhsT=wrm[:, 0:128], rhs=wrm[:],
                         start=True, stop=True)

    # ---- per-head SBUF tensors ----
    qm = [const.tile([65, T], F16, tag=f"qm{h}", name=f"qm{h}") for h in range(NH)]
    km = [const.tile([65, T], F16, tag=f"km{h}", name=f"km{h}") for h in range(NH)]
    qc = [const.tile([128, T], F16, tag=f"qc{h}", name=f"qc{h}") for h in range(NH)]
    kc = [const.tile([128, T], F16, tag=f"kc{h}", name=f"kc{h}") for h in range(NH)]
    vpx = [const.tile([128, DX], BF16, tag=f"vpx{t}", name=f"vpx{t}")
           for t in range(8)]
    yh = [const.tile([128, T], F16, tag=f"yh{g}", name=f"yh{g}") for g in range(2)]

    # ---- emission helpers; `weave` generators let one pass's matmuls be
    # interleaved into another pass's PE stream ----

    def proj_pass(wh, wl, xh, xl, dm, dc, hi_row, m, weave=None,
                  weave_every=3):
        """hi/lo projection for head pair m: 24 matmuls + evacuations.
        dm gets the fp16 hi part (rows 0:64); dc the [lo;hi] stack."""
        msl = slice(m * 128, (m + 1) * 128)
        ps = ps_big.tile([128, T], F32, tag="big", name="psb")
        n_mm = 0
        for n in range(2):
            nsl = slice(n * 512, (n + 1) * 512)
            mms = (
                [(wh[e], xl[e]) for e in range(4)]
                + [(wl[e], xh[e]) for e in range(4)]
                + [(wh[e], xh[e]) for e in range(4)]
            )
            for i_mm, (lw, rx) in enumerate(mms):
                nc.tensor.matmul(
                    ps[:, nsl],
                    lhsT=lw[:, msl],
                    rhs=rx[:, nsl],
                    start=(i_mm == 0), stop=(i_mm == len(mms) - 1),
                )
                n_mm += 1
                if weave is not None and n_mm % weave_every == 0:
                    next(weave, None)
        for hh in range(2):
            h = 2 * m + hh
            psl = slice(hh * 64, hh * 64 + 64)
            lo_row = 64 - hi_row
            # hi part (fp16 cast) -> K=65 "main" tile rows 0:64 (ACT)
            nc.scalar.copy(dm[h][0:64, :], ps[psl, :])
            # hi copy into the cross tile (ACT; proj phase has ACT slack)
            nc.scalar.copy(dc[h][hi_row:hi_row + 64, :], dm[h][0:64, :])
            # lo part = ps - hi  (DVE)
            nc.vector.tensor_sub(dc[h][lo_row:lo_row + 64, :], ps[psl, :],
                                 dm[h][0:64, :])

    def maxpass_gen(h, pool):
        """Generator: one next() emits one i-block (2 MMs + DVE reduce).
        `pool` is the PSUM pool the S tiles rotate through: ps_av during
        the projection phases (it is idle there, which keeps the weave
        from stalling the projection's long-lived ps_big accumulator),
        ps_big when woven into a mainpass.

        The (-max) column tile is turned into the fp16 bias row of qm[h]
        via PE transpose -> DVE cast -> one coalesced DMA.  (A strided
        DMA transpose degenerates to 4-byte packets and takes ~12us --
        measured; it throttled the whole mid-kernel in v1/v2.)"""
        colmax = const.tile([128, 8], F32, tag=f"cm{h}", name=f"cm{h}")
        for i in range(8):
            ps = pool.tile([128, T], F32, tag="av" if pool is ps_av else "big",
                           name="psm")
            for n in range(2):
                nsl = slice(n * 512, (n + 1) * 512)
                nc.tensor.matmul(
                    ps[:, nsl],
                    lhsT=qm[h][0:64, i * 128:(i + 1) * 128],
                    rhs=km[h][0:64, nsl],
                    start=True, stop=True,
                )
            nc.vector.reduce_max(colmax[:, i:i + 1], ps[:], axis=AX.X,
                                 negate=True)
            yield
        pst = pool.tile([8, 128], F32, tag="av" if pool is ps_av else "big",
                        name=f"pst{h}")
        nc.tensor.transpose(pst[:], colmax[:], ident[:])
        qmx = const.tile([8, 128], F16, tag=f"qmx{h}", name=f"qmx{h}")
        nc.scalar.copy(qmx[:], pst[:])
        sc = dramp.tile([8, 128], F16, tag=f"sc{h}", name=f"sc{h}")
        nc.sync.dma_start(sc[:], qmx[:])
        nc.sync.dma_start(qm[h][64:65, :], sc[:].rearrange("c p -> (c p)"))
        while True:
            yield

    def vp_pass():
        for t in range(8):
            ps = ps_big.tile([128, DX], F32, tag="big", name="psv")
            for e in range(4):
                nc.tensor.matmul(
                    ps[:],
                    lhsT=kvh_in[e][:, t * 128:(t + 1) * 128],
                    rhs=wv[e][:],
                    start=(e == 0), stop=(e == 3),
                )
            nc.scalar.copy(vpx[t][:], ps[:])
            for h4 in range(NH):
                c = h4 * (D + 1) + D
                nc.vector.memset(vpx[t][:, c:c + 1], 1.0)

    def norm_head(h, oex):
        """Normalize head h's AV result: yun = oex rows 0:64 (DVE evac),
        s = oex row 64; 1/s = exp(-ln s) on ACT (fp16 out, 5e-4 rel --
        well under the bf16 A-tile error).  The 64-partition broadcast is
        a rank-1 PE matmul ones_col^T @ recip into PSUM (a DMA bounce
        through DRAM costs 3-4us of latency here; the matmul ~0.5us).
        yh half = yun * recb (DVE, fp16 out -- values are V-scale after
        the normalize, fp16-safe)."""
        g, half = divmod(h, 2)
        # Heads 0-2 skip the [64,T] PSUM evacuation entirely: the final
        # multiply reads oex rows 0:64 straight from PSUM (gpsimd cannot
        # access PSUM, and DVE is the per-head pacer -- a separate copy
        # would cost another 1.2us there).  The oex slot is then held
        # until the mul, which the next head's lagged AV absorbs.  Head 3
        # keeps the evac on the idle-at-tail DVE so its mul can pair with
        # the PSUM-resident PE-broadcast recb.
        yun = oex[0:64, :]
        if h == 3:
            yun3 = const.tile([64, T], F32, tag="yun3", name="yun3")
            nc.vector.tensor_copy(yun3[:], oex[0:64, :])
            yun = yun3[:]
        lns = const.tile([1, T], F32, tag=f"ln{h}", name=f"ln{h}")
        nc.scalar.activation(lns[:], oex[64:65, :], AF.Ln)
        # f32: 1/s spans [e^-15, e^15] (the bias-row error window), which
        # overflows fp16.
        recip = const.tile([1, T], F32, tag=f"rcp{h}", name=f"rcp{h}")
        nc.scalar.activation(recip[:], lns[:], AF.Exp, scale=-1.0)
        if h == 3:
            # Tail head: broadcast via a K=1 f32 matmul into PSUM -- PE is
            # idle here and this cuts the DRAM round-trip off the final
            # serial chain.  (Mid-kernel this stalls the PE FIFO, so the
            # other heads use the DMA bounce instead.)
            recb = ps_big.tile([64, T], F32, tag="big", name=f"rcb{h}")
            for n in range(2):
                nsl = slice(n * 512, (n + 1) * 512)
                nc.tensor.matmul(recb[:, nsl], lhsT=ones_col[:],
                                 rhs=recip[:, nsl], start=True, stop=True)
            nc.vector.tensor_mul(yh[g][half * 64:(half + 1) * 64, :],
                                 yun, recb[:])
            return
        rrow = dramp.tile([1, T], F32, tag=f"rr{h}", name=f"rr{h}")
        nc.sync.dma_start(rrow[:], recip[:])
        recb = const.tile([64, T], F32, tag=f"rcb{h}", name=f"rcb{h}")
        rap = rrow[:]
        bcast = bass.AP(
            tensor=rap.tensor, offset=rap.offset,
            ap=[[0, 64]] + [list(d) for d in rap.ap[1:]],
        )
        nc.sync.dma_start(recb[:], bcast)
        nc.vector.tensor_mul(yh[g][half * 64:(half + 1) * 64, :], yun,
                             recb[:])

    def wo_block(g, i, copy_eng=None):
        pso = ps_big.tile([128, E], F32, tag="big", name="pso")
        nc.tensor.matmul(
            pso[:],
            lhsT=yh[g][:, i * 128:(i + 1) * 128],
            rhs=wo[g][:],
            start=True, stop=True,
        )
        osb = outp.tile([128, E], F32, tag="ot", name=f"ot{g}_{i}")
        if copy_eng == "scalar":
            nc.scalar.copy(osb[:], pso[:])
        else:
            nc.vector.tensor_copy(osb[:], pso[:])
        nc.sync.dma_start(dram[f"out{g}"][i * 128:(i + 1) * 128, :], osb[:])

    def mainpass(h, weave=None, wo_weave=None, norm_prev=None,
                 split_cross=False):
        """S^T - m = cross + main(bias row), exp (ACT, bf16), AV into oex.
        The AV matmuls lag one j-block behind S^T/exp so the PE FIFO is
        never parked on an AV that transitively waits on the previous
        head's normalize chain (oex slot WAR) or on ACT's exp cadence.
        `weave`: generator advanced once per step (maxpass of h+2).
        `wo_weave`: (g, [(step, i)...]) wo blocks emitted at given steps.
        `norm_prev`: head whose normalize chain rides this pass's start.
        `split_cross`: emit the cross matmul as two K=64 halves (exactly
        the same PSUM accumulation) -- pure PE-density padding so a pass
        with nothing woven in still keeps the HAM clock gate at 8/8."""
        oex = ps_av.tile([65, T], F32, tag="av", name="oex")
        ats = {}
        for j in range(9):
            if j < 8:
                jsl = slice(j * 128, (j + 1) * 128)
                ps = ps_big.tile([128, T], F32, tag="big", name="psb")
                for n in range(2):
                    nsl = slice(n * 512, (n + 1) * 512)
                    if split_cross:
                        nc.tensor.matmul(
                            ps[:, nsl], lhsT=kc[h][0:64, jsl],
                            rhs=qc[h][0:64, nsl], start=True, stop=False,
                        )
                        nc.tensor.matmul(
                            ps[:, nsl], lhsT=kc[h][64:128, jsl],
                            rhs=qc[h][64:128, nsl], start=False, stop=False,
                        )
                    else:
                        nc.tensor.matmul(
                            ps[:, nsl], lhsT=kc[h][:, jsl], rhs=qc[h][:, nsl],
                            start=True, stop=False,
                        )
                    nc.tensor.matmul(
                        ps[:, nsl], lhsT=km[h][:, jsl], rhs=qm[h][:, nsl],
                        start=False, stop=True,
                    )
                at = atp.tile([128, T], BF16, tag="at", name="at")
                nc.scalar.activation(at[:], ps[:], AF.Exp)
                ats[j] = at
            if j == 1 and norm_prev is not None:
                norm_head(*norm_prev)
            if j >= 1:
                ja = j - 1
                at = ats.pop(ja)
                for n in range(2):
                    nsl = slice(n * 512, (n + 1) * 512)
                    nc.tensor.matmul(
                        oex[:, nsl],
                        lhsT=vpx[ja][:, h * (D + 1):(h + 1) * (D + 1)],
                        rhs=at[:, nsl],
                        start=(ja == 0), stop=(ja == 7),
                    )
            if weave is not None:
                # Mildly front-loaded (2 blocks at steps 0-1, then 1 per
                # step; transpose/cast/DMA chain at step 6) so the NEXT
                # head's bias row lands before the pass boundary without
                # over-subscribing the psum ring early in the pass.
                next(weave, None)
                if j < 2:
                    next(weave, None)
            if wo_weave is not None:
                g, blocks = wo_weave
                for step, i in blocks:
                    if step == j:
                        wo_block(g, i)
        return oex

    # ---- emission schedule ----
    mx = [maxpass_gen(h, ps_big) for h in range(NH)]

    proj_pass(wqh, wql, qh_in, ql_in, qm, qc, 64, 0)             # q heads 0,1
    proj_pass(wkh, wkl, kvh_in, kvl_in, km, kc, 0, 0)            # k heads 0,1
    for h in range(NH):
        nc.vector.memset(km[h][64:65, :], 1.0)
    # maxpass(0) spreads over BOTH m=1 projection streams (1 i-block per
    # 6 proj MMs) so PE stays denser than the DVE reduce cadence; the
    # other heads' maxpasses ride the mainpasses one head ahead.
    proj_pass(wqh, wql, qh_in, ql_in, qm, qc, 64, 1,
              weave=mx[0], weave_every=6)                        # q heads 2,3
    proj_pass(wkh, wkl, kvh_in, kvl_in, km, kc, 0, 1,
              weave=mx[0], weave_every=6)                        # k heads 2,3
    next(mx[0], None)  # bias-row transpose/cast/DMA chain
    vp_pass()

    oex0 = mainpass(0, weave=mx[1])
    oex1 = mainpass(1, weave=mx[2], norm_prev=(0, oex0))
    oex2 = mainpass(2, weave=mx[3],
                    wo_weave=(0, [(j, j - 5) for j in range(5, 9)]),
                    norm_prev=(1, oex1))
    oex3 = mainpass(3, wo_weave=(0, [(j, j + 4) for j in range(0, 4)]),
                    norm_prev=(2, oex2))
    norm_head(3, oex3)
    for i in range(8):
        wo_block(1, i, copy_eng="scalar" if i % 2 else None)


class _Bacc(bacc.Bacc):
    """Bacc whose activation-table chooser can only pick the combined
    natural_log_exp_and_others set for Exp/Ln: the exp-only sets are
    blanked (entries kept so act_func_set_id indices stay aligned with
    act_info.json), which removes the per-head Exp<->Ln table swap
    (~2.7us each)."""

    def insert_act_table_loads(self):
        import bass_rust as _bass_rust
        from concourse.hw_specs import get_activation_tables
        has_activation = any(
            isinstance(i, mybir.InstActivation)
            for b in self.main_func.blocks
            for i in b.instructions
        )
        if not has_activation:
            return
        tables = []
        for name, fns in get_activation_tables(self.m.arch).items():
            if name in ("exp_and_others", "exp_and_friends"):
                fns = set()
            tables.append((name, fns))
        _bass_rust.insert_act_table_loads(self, tables)


def build_program():
    # Bacc (not raw Bass): its compile() splits multi-sem matmul waits onto
    # ldweights (TRN2 allows 1 wait/instruction) and lowers extended-ISA.
    nc = _Bacc("TRN2", target_bir_lowering=False, debug=False)
    dp = nc.declare_dram_parameter
    dram = {}
    for name in ("qh", "ql", "kvh", "kvl"):
        dram[name] = dp(name, [E, T], F16, isOutput=False)
    for name in ("wqh", "wql", "wkh", "wkl"):
        dram[name] = dp(name, [E, NH * D], F16, isOutput=False)
    dram["wv"] = dp("wv", [E, DX], F16, isOutput=False)
    dram["wo"] = dp("wo", [NH * D, E], F16, isOutput=False)
    dram["ident"] = dp("ident", [128, 128], F32, isOutput=False)
    dram["out0"] = dp("out0", [T, E], F32, isOutput=True)
    dram["out1"] = dp("out1", [T, E], F32, isOutput=True)
    with ExitStack() as ctx:
        tc = ctx.enter_context(tile.TileContext(nc))
        _emit(ctx, tc, dram)
    nc.finalize()  # Bacc.finalize runs compile() then freezes
    return nc


_PROGRAM = None


def _get_program():
    global _PROGRAM
    if _PROGRAM is None:
        _PROGRAM = build_program()
    return _PROGRAM


def _split16(x):
    h = x.astype(np.float16)
    l = (x - h.astype(np.float32)).astype(np.float16)
    return h, l


def make_in_maps(q, kv, Wq, Wk, Wv, Wo):
    in_maps = []
    for c in range(N_CORES):
        b, g = divmod(c, 2)
        heads = [g * NH + j for j in range(NH)]
        idx_q = [d * H + h for h in heads for d in range(D)]
        idx_k = [((D - d) % D) * H + h for h in heads for d in range(D)]
        qTh, qTl = _split16(np.ascontiguousarray(q[b].T))
        kvTh, kvTl = _split16(np.ascontiguousarray(kv[b].T))
        wq_h, wq_l = _split16(Wq[:, idx_q] * np.float32(SCALE))
        wk_h, wk_l = _split16(Wk[:, idx_k])
        wv_c = np.zeros((E, DX), np.float16)
        for j, h in enumerate(heads):
            wv_c[:, j * (D + 1):j * (D + 1) + D] = \
                Wv[:, [d * H + h for d in range(D)]].astype(np.float16)
        in_maps.append({
            "qh": qTh, "ql": qTl, "kvh": kvTh, "kvl": kvTl,
            "wqh": wq_h, "wql": wq_l, "wkh": wk_h, "wkl": wk_l,
            "wv": wv_c,
            "wo": Wo[g * NH * D:(g + 1) * NH * D, :].astype(np.float16),
            "ident": np.eye(128, dtype=np.float32),
        })
    return in_maps


def kernel(**inputs):
    global LAST_EXEC_NS
    q = np.asarray(inputs["q"], dtype=np.float32)
    kv = np.asarray(inputs["kv"], dtype=np.float32)
    Wq = np.asarray(inputs["Wq"], dtype=np.float32)
    Wk = np.asarray(inputs["Wk"], dtype=np.float32)
    Wv = np.asarray(inputs["Wv"], dtype=np.float32)
    Wo = np.asarray(inputs["Wo"], dtype=np.float32)
    B = q.shape[0]

    nc = _get_program()
    in_maps = make_in_maps(q, kv, Wq, Wk, Wv, Wo)
    res = run_bass_kernel_spmd(nc, in_maps, list(range(N_CORES)), trace=TRACE)
    LAST_EXEC_NS = res.exec_time_ns

    out = np.empty((B, T, E), np.float32)
    for b in range(B):
        r0, r1 = res.results[2 * b], res.results[2 * b + 1]
        out[b] = (r0["out0"] + r0["out1"]) + (r1["out0"] + r1["out1"])
    return out


# revision 43
# speedup vs baseline: 1.0386x; 1.0386x over previous
"""Trainium2 Bass kernel for MultiHeadFrequencyCrossAttention.

Math note: the reference computes, per (batch, head) slice,
    energy = ifft2( fft2(Q) @ fft2(K)^T * dk ).real
Because the DFT matrix F satisfies F @ F^T = n * P (P = index-negation
permutation), this collapses EXACTLY to
    energy = dk * D * Q @ K~^T        with K~[j, d] = K[j, (-d) mod D]
i.e. plain attention with K's head-dim index flipped (mod D) and an extra
scale of dk * D = 512.  No FFTs are needed; the flip and scale are folded
into host-side slices of the Wk / Wq projection weights.

Sharding: 8 cores = 4 batches x 2 head-groups (4 heads each).  Each core
gets q[b]^T, kv[b]^T (pre-transposed on host so the contraction dim lands
on SBUF partitions) plus its slice of the projection weights, computes
attention for its 4 heads, and applies its slice of Wo.  The host sums the
four partial Wo products per batch (two cores x two wo subgroups).

Precision scheme (PE fp32 matmuls are 4 cyc/row; fp16 is 1 cyc/row):
every value on the logit path is split hi/lo into two fp16 parts
(x = xh + xl, products of fp16 are exact in the fp32 PSUM accumulator), so
  x @ y ~= xh@yh + (xh@yl + xl@yh)     [~22-bit mantissa, err ~1e-6 rel]
One extra all-ones row in the stationary K operand times a "-rowmax" row
in the moving Q operand injects the softmax max-subtraction bias directly
into the S^T matmul.  The row max itself comes from a separate hi-only
fp16 pass (error ~ +-15 absolute on ~25000-scale logits, well inside the
exp() range window since A tiles are bf16).  A/V/output paths are plain
16-bit (error there stays relative, ~2e-3, no sharp-softmax blowup).

Engine assignment (v2 -- rebalanced so ACT only does exp/ln work and
gpsimd is not used at all, which avoids its library swaps):
  PE   : all matmuls (warmup, projections, max pass, S^T, AV, Wo)
  ACT  : A = exp(S^T - m) psum->sbuf, per-head ln(s) / exp(-ln s), and
         the proj hi evacuations (proj phase only, PE-bound there)
  DVE  : reduce_max (negated, fp16 out), proj lo subs, yun evacuation,
         y * (1/s) normalize (writes fp16 yh directly), wo psum->sbuf
  DMA  : inputs (chunked), colmax fp16 bounce straight into qm bias row,
         1/s row broadcast to 64 partitions (stride-0 descriptor),
         per-i-block output stores (separate out0/out1, host adds)

Emission order software-pipelines everything: maxpass(0)/(1) matmuls are
woven into the m=1 projection streams, maxpass(2)/(3) into mainpass(0)/(1)
j-blocks, wo(0) into mainpass(2), so PE never idles long enough for the
HAM clock gate to re-throttle (the v1 kernel spent 110us at half clock).
"""

import numpy as np
from contextlib import ExitStack

import concourse.bass as bass
import concourse.tile as tile
from concourse import bacc, mybir
from concourse.bass_utils import run_bass_kernel_spmd

F32 = mybir.dt.float32
F16 = mybir.dt.float16
BF16 = mybir.dt.bfloat16
AX = mybir.AxisListType
AF = mybir.ActivationFunctionType

T = 1024          # sequence length
E = 512           # embed dim
H = 8             # total heads
D = E // H        # head dim = 64
NH = 4            # heads per core
DX = NH * (D + 1) # vp columns incl. ones = 260
N_CORES = 8
SCALE = float(D) * float(D) ** 0.5  # dk * D = 512.0

TRACE = False          # set by test harness; adds NTFF profiling
LAST_EXEC_NS = None


def _emit(ctx, tc, dram):
    nc = tc.nc
    const = ctx.enter_context(tc.tile_pool(name="const", bufs=1))
    ps_big = ctx.enter_context(tc.tile_pool(name="ps_big", bufs=3, space="PSUM"))
    ps_av = ctx.enter_context(tc.tile_pool(name="ps_av", bufs=1, space="PSUM"))
    atp = ctx.enter_context(tc.tile_pool(name="atp", bufs=4))
    outp = ctx.enter_context(tc.tile_pool(name="outp", bufs=4))
    dramp = ctx.enter_context(tc.tile_pool(name="dramp", bufs=1, space="DRAM"))

    # ---- input loads (all fp16 on the wire) ----
    # The four big [E, T] tensors are split into T-halves so the first
    # projection block can start after ~1.5 MB instead of ~2.5 MB.
    tiles3 = {}
    def declare(name, cols):
        t3 = const.tile([128, 4, cols], F16, tag=name, name=name)
        tiles3[name] = t3
        return [t3[:, e, :] for e in range(4)]

    def load_half(name, half, eng=None):
        t3 = tiles3[name]
        cols = t3.shape[2]
        h0, h1 = half * cols // 2, (half + 1) * cols // 2
        (eng or nc.sync).dma_start(
            t3[:, :, h0:h1],
            dram[name][:, h0:h1].rearrange("(c p) t -> p c t", p=128),
        )

    def load_full(name, eng=None):
        t3 = tiles3[name]
        (eng or nc.sync).dma_start(
            t3[:], dram[name][:].rearrange("(c p) t -> p c t", p=128)
        )

    wqh = declare("wqh", NH * D)
    wql = declare("wql", NH * D)
    qh_in = declare("qh", T)
    ql_in = declare("ql", T)
    wkh = declare("wkh", NH * D)
    wkl = declare("wkl", NH * D)
    kvh_in = declare("kvh", T)
    kvl_in = declare("kvl", T)
    wv = declare("wv", DX)
    wo3 = const.tile([128, 2, E], F16, tag="wo", name="wo")
    wo = [wo3[:, g, :] for g in range(2)]
    ident = const.tile([128, 128], F32, tag="ident", name="ident")
    nc.sync.dma_start(ident[:], dram["ident"][:])

    # q-proj n=0 needs the q tensors first; k/v loads ride the gpsimd
    # DGE queue so the two streams run on separate hardware queues.
    load_full("wqh"); load_full("wql")
    load_half("ql", 0); load_half("qh", 0)
    load_full("wkh", nc.gpsimd); load_full("wkl", nc.gpsimd)
    load_half("kvl", 0, nc.gpsimd); load_half("kvh", 0, nc.gpsimd)
    load_half("ql", 1); load_half("qh", 1)
    load_half("kvl", 1, nc.gpsimd); load_half("kvh", 1, nc.gpsimd)
    load_full("wv", nc.gpsimd)
    nc.sync.dma_start(wo3[:], dram["wo"][:].rearrange("(g p) t -> p g t", p=128))

    # PE warm-up: dummy matmuls fill the input-DMA window so the HAM clock
    # gate is already at 8/8 (2.4 GHz) when the projections start.  18 MMs
    # x ~430ns cold spans the ~8us until the first projection operands land.
    wrm = const.tile([128, 512], F16, tag="wrm", name="wrm")
    nc.vector.memset(wrm[:], 0.0)
    ones_col = const.tile([1, 64], BF16, tag="ones_col", name="ones_col")
    nc.vector.memset(ones_col[:], 1.0)
    # Dummy Exp/Ln so the natural_log_exp table set loads inside the input
    # DMA window instead of stalling ACT at mainpass(0)'s first exp.
    dtab = const.tile([1, 2], F32, tag="dtab", name="dtab")
    nc.scalar.activation(dtab[:], wrm[0:1, 0:2], AF.Exp)
    nc.scalar.activation(dtab[:], wrm[0:1, 0:2], AF.Ln)
    for w in range(18):
        pw = ps_big.tile([128, 512], F32, tag="big", name="psw")
        nc.tensor.matmul(pw[:], lhsT=wrm[:, 0:128], rhs=wrm[:],
                         start=True, stop=True)

    # ---- per-head SBUF tensors ----
    qm = [const.tile([65, T], F16, tag=f"qm{h}", name=f"qm{h}") for h in range(NH)]
    km = [const.tile([65, T], F16, tag=f"km{h}", name=f"km{h}") for h in range(NH)]
    qc = [const.tile([128, T], F16, tag=f"qc{h}", name=f"qc{h}") for h in range(NH)]
    kc = [const.tile([128, T], F16, tag=f"kc{h}", name=f"kc{h}") for h in range(NH)]
    vpx = [const.tile([128, DX], BF16, tag=f"vpx{t}", name=f"vpx{t}")
           for t in range(8)]
    yh = [const.tile([128, T], F16, tag=f"yh{g}", name=f"yh{g}") for g in range(2)]

    # ---- emission helpers; `weave` generators let one pass's matmuls be
    # interleaved into another pass's PE stream ----

    def proj_pass(wh, wl, xh, xl, dm, dc, hi_row, m, weave=None,
                  weave_every=3):
        """hi/lo projection for head pair m: 24 matmuls + evacuations.
        dm gets the fp16 hi part (rows 0:64); dc the [lo;hi] stack."""
        msl = slice(m * 128, (m + 1) * 128)
        ps = ps_big.tile([128, T], F32, tag="big", name="psb")
        n_mm = 0
        for n in range(2):
            nsl = slice(n * 512, (n + 1) * 512)
            mms = (
                [(wh[e], xl[e]) for e in range(4)]
                + [(wl[e], xh[e]) for e in range(4)]
                + [(wh[e], xh[e]) for e in range(4)]
            )
            for i_mm, (lw, rx) in enumerate(mms):
                nc.tensor.matmul(
                    ps[:, nsl],
                    lhsT=lw[:, msl],
                    rhs=rx[:, nsl],
                    start=(i_mm == 0), stop=(i_mm == len(mms) - 1),
                )
                n_mm += 1
                if weave is not None and n_mm % weave_every == 0:
                    next(weave, None)
        for hh in range(2):
            h = 2 * m + hh
            psl = slice(hh * 64, hh * 64 + 64)
            lo_row = 64 - hi_row
            # hi part (fp16 cast) -> K=65 "main" tile rows 0:64 (ACT)
            nc.scalar.copy(dm[h][0:64, :], ps[psl, :])
            # hi copy into the cross tile (ACT; proj phase has ACT slack)
            nc.scalar.copy(dc[h][hi_row:hi_row + 64, :], dm[h][0:64, :])
            # lo part = ps - hi  (DVE)
            nc.vector.tensor_sub(dc[h][lo_row:lo_row + 64, :], ps[psl, :],
                                 dm[h][0:64, :])

    def maxpass_gen(h, pool):
        """Generator: one next() emits one i-block (2 MMs + DVE reduce).
        `pool` is the PSUM pool the S tiles rotate through: ps_av during
        the projection phases (it is idle there, which keeps the weave
        from stalling the projection's long-lived ps_big accumulator),
        ps_big when woven into a mainpass.

        The (-max) column tile is turned into the fp16 bias row of qm[h]
        via PE transpose -> DVE cast -> one coalesced DMA.  (A strided
        DMA transpose degenerates to 4-byte packets and takes ~12us --
        measured; it throttled the whole mid-kernel in v1/v2.)"""
        colmax = const.tile([128, 8], F32, tag=f"cm{h}", name=f"cm{h}")
        for i in range(8):
            ps = pool.tile([128, T], F32, tag="av" if pool is ps_av else "big",
                           name="psm")
            for n in range(2):
                nsl = slice(n * 512, (n + 1) * 512)
                nc.tensor.matmul(
                    ps[:, nsl],
                    lhsT=qm[h][0:64, i * 128:(i + 1) * 128],
                    rhs=km[h][0:64, nsl],
                    start=True, stop=True,
                )
            nc.vector.reduce_max(colmax[:, i:i + 1], ps[:], axis=AX.X,
                                 negate=True)
            yield
        pst = pool.tile([8, 128], F32, tag="av" if pool is ps_av else "big",
                        name=f"pst{h}")
        nc.tensor.transpose(pst[:], colmax[:], ident[:])
        qmx = const.tile([8, 128], F16, tag=f"qmx{h}", name=f"qmx{h}")
        nc.scalar.copy(qmx[:], pst[:])
        sc = dramp.tile([8, 128], F16, tag=f"sc{h}", name=f"sc{h}")
        nc.sync.dma_start(sc[:], qmx[:])
        nc.sync.dma_start(qm[h][64:65, :], sc[:].rearrange("c p -> (c p)"))
        while True:
            yield

    def vp_pass():
        for t in range(8):
            ps = ps_big.tile([128, DX], F32, tag="big", name="psv")
            for e in range(4):
                nc.tensor.matmul(
                    ps[:],
                    lhsT=kvh_in[e][:, t * 128:(t + 1) * 128],
                    rhs=wv[e][:],
                    start=(e == 0), stop=(e == 3),
                )
            nc.scalar.copy(vpx[t][:], ps[:])
            for h4 in range(NH):
                c = h4 * (D + 1) + D
                nc.vector.memset(vpx[t][:, c:c + 1], 1.0)

    def norm_head(h, oex):
        """Normalize head h's AV result: yun = oex rows 0:64 (DVE evac),
        s = oex row 64; 1/s = exp(-ln s) on ACT (fp16 out, 5e-4 rel --
        well under the bf16 A-tile error).  The 64-partition broadcast is
        a rank-1 PE matmul ones_col^T @ recip into PSUM (a DMA bounce
        through DRAM costs 3-4us of latency here; the matmul ~0.5us).
        yh half = yun * recb (DVE, fp16 out -- values are V-scale after
        the normalize, fp16-safe)."""
        g, half = divmod(h, 2)
        # Heads 0-2 skip the [64,T] PSUM evacuation entirely: the final
        # multiply reads oex rows 0:64 straight from PSUM (gpsimd cannot
        # access PSUM, and DVE is the per-head pacer -- a separate copy
        # would cost another 1.2us there).  The oex slot is then held
        # until the mul, which the next head's lagged AV absorbs.  Head 3
        # keeps the evac on the idle-at-tail DVE so its mul can pair with
        # the PSUM-resident PE-broadcast recb.
        yun = oex[0:64, :]
        if h == 3:
            yun3 = const.tile([64, T], F32, tag="yun3", name="yun3")
            nc.vector.tensor_copy(yun3[:], oex[0:64, :])
            yun = yun3[:]
        lns = const.tile([1, T], F32, tag=f"ln{h}", name=f"ln{h}")
        nc.scalar.activation(lns[:], oex[64:65, :], AF.Ln)
        if h == 3:
            # Tail head: broadcast via a K=1 matmul into PSUM -- PE is
            # idle here and this cuts the DRAM round-trip off the final
            # serial chain.  (Mid-kernel this stalls the PE FIFO, so the
            # other heads use the DMA bounce instead.)  bf16 recip: f32
            # matmuls are 4 passes/col (~4us measured for the pair); bf16
            # keeps f32 RANGE (1/s spans e^+-15) and its 2^-9 rounding
            # only touches head 3's quarter of the output (~0.005 abs).
            recip_b = const.tile([1, T], BF16, tag="rcpb", name="rcpb")
            nc.scalar.activation(recip_b[:], lns[:], AF.Exp, scale=-1.0)
            recb = ps_big.tile([64, T], F32, tag="big", name=f"rcb{h}")
            for n in range(2):
                nsl = slice(n * 512, (n + 1) * 512)
                nc.tensor.matmul(recb[:, nsl], lhsT=ones_col[:],
                                 rhs=recip_b[:, nsl], start=True, stop=True)
            nc.vector.tensor_mul(yh[g][half * 64:(half + 1) * 64, :],
                                 yun, recb[:])
            return
        # f32: 1/s spans [e^-15, e^15] (the bias-row error window), which
        # overflows fp16; the DMA broadcast doesn't care about dtype.
        recip = const.tile([1, T], F32, tag=f"rcp{h}", name=f"rcp{h}")
        nc.scalar.activation(recip[:], lns[:], AF.Exp, scale=-1.0)
        rrow = dramp.tile([1, T], F32, tag=f"rr{h}", name=f"rr{h}")
        nc.sync.dma_start(rrow[:], recip[:])
        recb = const.tile([64, T], F32, tag=f"rcb{h}", name=f"rcb{h}")
        rap = rrow[:]
        bcast = bass.AP(
            tensor=rap.tensor, offset=rap.offset,
            ap=[[0, 64]] + [list(d) for d in rap.ap[1:]],
        )
        nc.sync.dma_start(recb[:], bcast)
        nc.vector.tensor_mul(yh[g][half * 64:(half + 1) * 64, :], yun,
                             recb[:])

    def wo_block(g, i, copy_eng=None):
        pso = ps_big.tile([128, E], F32, tag="big", name="pso")
        nc.tensor.matmul(
            pso[:],
            lhsT=yh[g][:, i * 128:(i + 1) * 128],
            rhs=wo[g][:],
            start=True, stop=True,
        )
        osb = outp.tile([128, E], F32, tag="ot", name=f"ot{g}_{i}")
        if copy_eng == "scalar":
            nc.scalar.copy(osb[:], pso[:])
        else:
            nc.vector.tensor_copy(osb[:], pso[:])
        nc.sync.dma_start(dram[f"out{g}"][i * 128:(i + 1) * 128, :], osb[:])

    def mainpass(h, weave=None, wo_weave=None, norm_prev=None,
                 split_cross=False):
        """S^T - m = cross + main(bias row), exp (ACT, bf16), AV into oex.
        The AV matmuls lag one j-block behind S^T/exp so the PE FIFO is
        never parked on an AV that transitively waits on the previous
        head's normalize chain (oex slot WAR) or on ACT's exp cadence.
        `weave`: generator advanced once per step (maxpass of h+2).
        `wo_weave`: (g, [(step, i)...]) wo blocks emitted at given steps.
        `norm_prev`: head whose normalize chain rides this pass's start.
        `split_cross`: emit the cross matmul as two K=64 halves (exactly
        the same PSUM accumulation) -- pure PE-density padding so a pass
        with nothing woven in still keeps the HAM clock gate at 8/8."""
        oex = ps_av.tile([65, T], F32, tag="av", name="oex")
        ats = {}
        for j in range(9):
            if j < 8:
                jsl = slice(j * 128, (j + 1) * 128)
                ps = ps_big.tile([128, T], F32, tag="big", name="psb")
                for n in range(2):
                    nsl = slice(n * 512, (n + 1) * 512)
                    if split_cross:
                        nc.tensor.matmul(
                            ps[:, nsl], lhsT=kc[h][0:64, jsl],
                            rhs=qc[h][0:64, nsl], start=True, stop=False,
                        )
                        nc.tensor.matmul(
                            ps[:, nsl], lhsT=kc[h][64:128, jsl],
                            rhs=qc[h][64:128, nsl], start=False, stop=False,
                        )
                    else:
                        nc.tensor.matmul(
                            ps[:, nsl], lhsT=kc[h][:, jsl], rhs=qc[h][:, nsl],
                            start=True, stop=False,
                        )
                    nc.tensor.matmul(
                        ps[:, nsl], lhsT=km[h][:, jsl], rhs=qm[h][:, nsl],
                        start=False, stop=True,
                    )
                at = atp.tile([128, T], BF16, tag="at", name="at")
                nc.scalar.activation(at[:], ps[:], AF.Exp)
                ats[j] = at
            if j == 1 and norm_prev is not None:
                norm_head(*norm_prev)
            if j >= 1:
                ja = j - 1
                at = ats.pop(ja)
                for n in range(2):
                    nsl = slice(n * 512, (n + 1) * 512)
                    nc.tensor.matmul(
                        oex[:, nsl],
                        lhsT=vpx[ja][:, h * (D + 1):(h + 1) * (D + 1)],
                        rhs=at[:, nsl],
                        start=(ja == 0), stop=(ja == 7),
                    )
            if weave is not None:
                # Mildly front-loaded (2 blocks at steps 0-1, then 1 per
                # step; transpose/cast/DMA chain at step 6) so the NEXT
                # head's bias row lands before the pass boundary without
                # over-subscribing the psum ring early in the pass.
                next(weave, None)
                if j < 2:
                    next(weave, None)
            if wo_weave is not None:
                g, blocks = wo_weave
                for step, i in blocks:
                    if step == j:
                        wo_block(g, i, copy_eng="scalar" if i % 2 else None)
        return oex

    # ---- emission schedule ----
    mx = [maxpass_gen(h, ps_big) for h in range(NH)]

    proj_pass(wqh, wql, qh_in, ql_in, qm, qc, 64, 0)             # q heads 0,1
    proj_pass(wkh, wkl, kvh_in, kvl_in, km, kc, 0, 0)            # k heads 0,1
    for h in range(NH):
        nc.vector.memset(km[h][64:65, :], 1.0)
    # maxpass(0) spreads over BOTH m=1 projection streams (1 i-block per
    # 6 proj MMs) so PE stays denser than the DVE reduce cadence; the
    # other heads' maxpasses ride the mainpasses one head ahead.
    proj_pass(wqh, wql, qh_in, ql_in, qm, qc, 64, 1,
              weave=mx[0], weave_every=6)                        # q heads 2,3
    proj_pass(wkh, wkl, kvh_in, kvl_in, km, kc, 0, 1,
              weave=mx[0], weave_every=6)                        # k heads 2,3
    next(mx[0], None)  # bias-row transpose/cast/DMA chain
    vp_pass()

    oex0 = mainpass(0, weave=mx[1])
    oex1 = mainpass(1, weave=mx[2], norm_prev=(0, oex0))
    oex2 = mainpass(2, weave=mx[3],
                    wo_weave=(0, [(j, j - 5) for j in range(5, 9)]),
                    norm_prev=(1, oex1))
    oex3 = mainpass(3, wo_weave=(0, [(j, j + 4) for j in range(0, 4)]),
                    norm_prev=(2, oex2))
    norm_head(3, oex3)
    for i in range(8):
        wo_block(1, i, copy_eng="scalar" if i % 2 else None)


class _Bacc(bacc.Bacc):
    """Bacc whose activation-table chooser can only pick the combined
    natural_log_exp_and_others set for Exp/Ln: the exp-only sets are
    blanked (entries kept so act_func_set_id indices stay aligned with
    act_info.json), which removes the per-head Exp<->Ln table swap
    (~2.7us each)."""

    def insert_act_table_loads(self):
        import bass_rust as _bass_rust
        from concourse.hw_specs import get_activation_tables
        has_activation = any(
            isinstance(i, mybir.InstActivation)
            for b in self.main_func.blocks
            for i in b.instructions
        )
        if not has_activation:
            return
        tables = []
        for name, fns in get_activation_tables(self.m.arch).items():
            if name in ("exp_and_others", "exp_and_friends"):
                fns = set()
            tables.append((name, fns))
        _bass_rust.insert_act_table_loads(self, tables)


def build_program():
    # Bacc (not raw Bass): its compile() splits multi-sem matmul waits onto
    # ldweights (TRN2 allows 1 wait/instruction) and lowers extended-ISA.
    nc = _Bacc("TRN2", target_bir_lowering=False, debug=False)
    dp = nc.declare_dram_parameter
    dram = {}
    for name in ("qh", "ql", "kvh", "kvl"):
        dram[name] = dp(name, [E, T], F16, isOutput=False)
    for name in ("wqh", "wql", "wkh", "wkl"):
        dram[name] = dp(name, [E, NH * D], F16, isOutput=False)
    dram["wv"] = dp("wv", [E, DX], F16, isOutput=False)
    dram["wo"] = dp("wo", [NH * D, E], F16, isOutput=False)
    dram["ident"] = dp("ident", [128, 128], F32, isOutput=False)
    dram["out0"] = dp("out0", [T, E], F32, isOutput=True)
    dram["out1"] = dp("out1", [T, E], F32, isOutput=True)
    with ExitStack() as ctx:
        tc = ctx.enter_context(tile.TileContext(nc))
        _emit(ctx, tc, dram)
    nc.finalize()  # Bacc.finalize runs compile() then freezes
    return nc


_PROGRAM = None


def _get_program():
    global _PROGRAM
    if _PROGRAM is None:
        _PROGRAM = build_program()
    return _PROGRAM


def _split16(x):
    h = x.astype(np.float16)
    l = (x - h.astype(np.float32)).astype(np.float16)
    return h, l


def make_in_maps(q, kv, Wq, Wk, Wv, Wo):
    in_maps = []
    for c in range(N_CORES):
        b, g = divmod(c, 2)
        heads = [g * NH + j for j in range(NH)]
        idx_q = [d * H + h for h in heads for d in range(D)]
        idx_k = [((D - d) % D) * H + h for h in heads for d in range(D)]
        qTh, qTl = _split16(np.ascontiguousarray(q[b].T))
        kvTh, kvTl = _split16(np.ascontiguousarray(kv[b].T))
        wq_h, wq_l = _split16(Wq[:, idx_q] * np.float32(SCALE))
        wk_h, wk_l = _split16(Wk[:, idx_k])
        wv_c = np.zeros((E, DX), np.float16)
        for j, h in enumerate(heads):
            wv_c[:, j * (D + 1):j * (D + 1) + D] = \
                Wv[:, [d * H + h for d in range(D)]].astype(np.float16)
        in_maps.append({
            "qh": qTh, "ql": qTl, "kvh": kvTh, "kvl": kvTl,
            "wqh": wq_h, "wql": wq_l, "wkh": wk_h, "wkl": wk_l,
            "wv": wv_c,
            "wo": Wo[g * NH * D:(g + 1) * NH * D, :].astype(np.float16),
            "ident": np.eye(128, dtype=np.float32),
        })
    return in_maps


def kernel(**inputs):
    global LAST_EXEC_NS
    q = np.asarray(inputs["q"], dtype=np.float32)
    kv = np.asarray(inputs["kv"], dtype=np.float32)
    Wq = np.asarray(inputs["Wq"], dtype=np.float32)
    Wk = np.asarray(inputs["Wk"], dtype=np.float32)
    Wv = np.asarray(inputs["Wv"], dtype=np.float32)
    Wo = np.asarray(inputs["Wo"], dtype=np.float32)
    B = q.shape[0]

    nc = _get_program()
    in_maps = make_in_maps(q, kv, Wq, Wk, Wv, Wo)
    res = run_bass_kernel_spmd(nc, in_maps, list(range(N_CORES)), trace=TRACE)
    LAST_EXEC_NS = res.exec_time_ns

    out = np.empty((B, T, E), np.float32)
    for b in range(B):
        r0, r1 = res.results[2 * b], res.results[2 * b + 1]
        out[b] = (r0["out0"] + r0["out1"]) + (r1["out0"] + r1["out1"])
    return out


# revision 44
# speedup vs baseline: 1.0398x; 1.0012x over previous
"""Trainium2 Bass kernel for MultiHeadFrequencyCrossAttention.

Math note: the reference computes, per (batch, head) slice,
    energy = ifft2( fft2(Q) @ fft2(K)^T * dk ).real
Because the DFT matrix F satisfies F @ F^T = n * P (P = index-negation
permutation), this collapses EXACTLY to
    energy = dk * D * Q @ K~^T        with K~[j, d] = K[j, (-d) mod D]
i.e. plain attention with K's head-dim index flipped (mod D) and an extra
scale of dk * D = 512.  No FFTs are needed; the flip and scale are folded
into host-side slices of the Wk / Wq projection weights.

Sharding: 8 cores = 4 batches x 2 head-groups (4 heads each).  Each core
gets q[b]^T, kv[b]^T (pre-transposed on host so the contraction dim lands
on SBUF partitions) plus its slice of the projection weights, computes
attention for its 4 heads, and applies its slice of Wo.  The host sums the
four partial Wo products per batch (two cores x two wo subgroups).

Precision scheme (PE fp32 matmuls are 4 cyc/row; fp16 is 1 cyc/row):
every value on the logit path is split hi/lo into two fp16 parts
(x = xh + xl, products of fp16 are exact in the fp32 PSUM accumulator), so
  x @ y ~= xh@yh + (xh@yl + xl@yh)     [~22-bit mantissa, err ~1e-6 rel]
One extra all-ones row in the stationary K operand times a "-rowmax" row
in the moving Q operand injects the softmax max-subtraction bias directly
into the S^T matmul.  The row max itself comes from a separate hi-only
fp16 pass (error ~ +-15 absolute on ~25000-scale logits, well inside the
exp() range window since A tiles are bf16).  A/V/output paths are plain
16-bit (error there stays relative, ~2e-3, no sharp-softmax blowup).

Engine assignment (v2 -- rebalanced so ACT only does exp/ln work and
gpsimd is not used at all, which avoids its library swaps):
  PE   : all matmuls (warmup, projections, max pass, S^T, AV, Wo)
  ACT  : A = exp(S^T - m) psum->sbuf, per-head ln(s) / exp(-ln s), and
         the proj hi evacuations (proj phase only, PE-bound there)
  DVE  : reduce_max (negated, fp16 out), proj lo subs, yun evacuation,
         y * (1/s) normalize (writes fp16 yh directly), wo psum->sbuf
  DMA  : inputs (chunked), colmax fp16 bounce straight into qm bias row,
         1/s row broadcast to 64 partitions (stride-0 descriptor),
         per-i-block output stores (separate out0/out1, host adds)

Emission order software-pipelines everything: maxpass(0)/(1) matmuls are
woven into the m=1 projection streams, maxpass(2)/(3) into mainpass(0)/(1)
j-blocks, wo(0) into mainpass(2), so PE never idles long enough for the
HAM clock gate to re-throttle (the v1 kernel spent 110us at half clock).
"""

import numpy as np
from contextlib import ExitStack

import concourse.bass as bass
import concourse.tile as tile
from concourse import bacc, mybir
from concourse.bass_utils import run_bass_kernel_spmd

F32 = mybir.dt.float32
F16 = mybir.dt.float16
BF16 = mybir.dt.bfloat16
AX = mybir.AxisListType
AF = mybir.ActivationFunctionType

T = 1024          # sequence length
E = 512           # embed dim
H = 8             # total heads
D = E // H        # head dim = 64
NH = 4            # heads per core
DX = NH * (D + 1) # vp columns incl. ones = 260
N_CORES = 8
SCALE = float(D) * float(D) ** 0.5  # dk * D = 512.0

TRACE = False          # set by test harness; adds NTFF profiling
LAST_EXEC_NS = None


def _emit(ctx, tc, dram):
    nc = tc.nc
    const = ctx.enter_context(tc.tile_pool(name="const", bufs=1))
    ps_big = ctx.enter_context(tc.tile_pool(name="ps_big", bufs=3, space="PSUM"))
    ps_av = ctx.enter_context(tc.tile_pool(name="ps_av", bufs=1, space="PSUM"))
    atp = ctx.enter_context(tc.tile_pool(name="atp", bufs=4))
    outp = ctx.enter_context(tc.tile_pool(name="outp", bufs=4))
    dramp = ctx.enter_context(tc.tile_pool(name="dramp", bufs=1, space="DRAM"))

    # ---- input loads (all fp16 on the wire) ----
    # The four big [E, T] tensors are split into T-halves so the first
    # projection block can start after ~1.5 MB instead of ~2.5 MB.
    tiles3 = {}
    def declare(name, cols):
        t3 = const.tile([128, 4, cols], F16, tag=name, name=name)
        tiles3[name] = t3
        return [t3[:, e, :] for e in range(4)]

    def load_half(name, half, eng=None):
        t3 = tiles3[name]
        cols = t3.shape[2]
        h0, h1 = half * cols // 2, (half + 1) * cols // 2
        (eng or nc.sync).dma_start(
            t3[:, :, h0:h1],
            dram[name][:, h0:h1].rearrange("(c p) t -> p c t", p=128),
        )

    def load_full(name, eng=None):
        t3 = tiles3[name]
        (eng or nc.sync).dma_start(
            t3[:], dram[name][:].rearrange("(c p) t -> p c t", p=128)
        )

    wqh = declare("wqh", NH * D)
    wql = declare("wql", NH * D)
    qh_in = declare("qh", T)
    ql_in = declare("ql", T)
    wkh = declare("wkh", NH * D)
    wkl = declare("wkl", NH * D)
    kvh_in = declare("kvh", T)
    kvl_in = declare("kvl", T)
    wv = declare("wv", DX)
    wo3 = const.tile([128, 2, E], F16, tag="wo", name="wo")
    wo = [wo3[:, g, :] for g in range(2)]
    ident = const.tile([128, 128], F32, tag="ident", name="ident")
    nc.sync.dma_start(ident[:], dram["ident"][:])

    # One queue, strict priority order (two queues just split the HBM
    # bandwidth and delay the critical q-proj operands -- measured).
    load_full("wqh"); load_full("wql")
    load_half("ql", 0); load_half("qh", 0)
    load_half("ql", 1); load_half("qh", 1)
    load_full("wkh"); load_full("wkl")
    load_half("kvl", 0); load_half("kvh", 0)
    load_half("kvl", 1); load_half("kvh", 1)
    load_full("wv")
    nc.sync.dma_start(wo3[:], dram["wo"][:].rearrange("(g p) t -> p g t", p=128))

    # PE warm-up: dummy matmuls fill the input-DMA window so the HAM clock
    # gate is already at 8/8 (2.4 GHz) when the projections start.  18 MMs
    # x ~430ns cold spans the ~8us until the first projection operands land.
    wrm = const.tile([128, 512], F16, tag="wrm", name="wrm")
    nc.vector.memset(wrm[:], 0.0)
    ones_col = const.tile([1, 64], BF16, tag="ones_col", name="ones_col")
    nc.vector.memset(ones_col[:], 1.0)
    # Dummy Exp/Ln so the natural_log_exp table set loads inside the input
    # DMA window instead of stalling ACT at mainpass(0)'s first exp.
    dtab = const.tile([1, 2], F32, tag="dtab", name="dtab")
    nc.scalar.activation(dtab[:], wrm[0:1, 0:2], AF.Exp)
    nc.scalar.activation(dtab[:], wrm[0:1, 0:2], AF.Ln)
    for w in range(18):
        pw = ps_big.tile([128, 512], F32, tag="big", name="psw")
        nc.tensor.matmul(pw[:], lhsT=wrm[:, 0:128], rhs=wrm[:],
                         start=True, stop=True)

    # ---- per-head SBUF tensors ----
    qm = [const.tile([65, T], F16, tag=f"qm{h}", name=f"qm{h}") for h in range(NH)]
    km = [const.tile([65, T], F16, tag=f"km{h}", name=f"km{h}") for h in range(NH)]
    qc = [const.tile([128, T], F16, tag=f"qc{h}", name=f"qc{h}") for h in range(NH)]
    kc = [const.tile([128, T], F16, tag=f"kc{h}", name=f"kc{h}") for h in range(NH)]
    vpx = [const.tile([128, DX], BF16, tag=f"vpx{t}", name=f"vpx{t}")
           for t in range(8)]
    yh = [const.tile([128, T], F16, tag=f"yh{g}", name=f"yh{g}") for g in range(2)]

    # ---- emission helpers; `weave` generators let one pass's matmuls be
    # interleaved into another pass's PE stream ----

    def proj_pass(wh, wl, xh, xl, dm, dc, hi_row, m, weave=None,
                  weave_every=3):
        """hi/lo projection for head pair m: 24 matmuls + evacuations.
        dm gets the fp16 hi part (rows 0:64); dc the [lo;hi] stack."""
        msl = slice(m * 128, (m + 1) * 128)
        ps = ps_big.tile([128, T], F32, tag="big", name="psb")
        n_mm = 0
        for n in range(2):
            nsl = slice(n * 512, (n + 1) * 512)
            mms = (
                [(wh[e], xl[e]) for e in range(4)]
                + [(wl[e], xh[e]) for e in range(4)]
                + [(wh[e], xh[e]) for e in range(4)]
            )
            for i_mm, (lw, rx) in enumerate(mms):
                nc.tensor.matmul(
                    ps[:, nsl],
                    lhsT=lw[:, msl],
                    rhs=rx[:, nsl],
                    start=(i_mm == 0), stop=(i_mm == len(mms) - 1),
                )
                n_mm += 1
                if weave is not None and n_mm % weave_every == 0:
                    next(weave, None)
        for hh in range(2):
            h = 2 * m + hh
            psl = slice(hh * 64, hh * 64 + 64)
            lo_row = 64 - hi_row
            # hi part (fp16 cast) -> K=65 "main" tile rows 0:64 (ACT)
            nc.scalar.copy(dm[h][0:64, :], ps[psl, :])
            # hi copy into the cross tile (ACT; proj phase has ACT slack)
            nc.scalar.copy(dc[h][hi_row:hi_row + 64, :], dm[h][0:64, :])
            # lo part = ps - hi  (DVE)
            nc.vector.tensor_sub(dc[h][lo_row:lo_row + 64, :], ps[psl, :],
                                 dm[h][0:64, :])

    def maxpass_gen(h, pool):
        """Generator: one next() emits one i-block (2 MMs + DVE reduce).
        `pool` is the PSUM pool the S tiles rotate through: ps_av during
        the projection phases (it is idle there, which keeps the weave
        from stalling the projection's long-lived ps_big accumulator),
        ps_big when woven into a mainpass.

        The (-max) column tile is turned into the fp16 bias row of qm[h]
        via PE transpose -> DVE cast -> one coalesced DMA.  (A strided
        DMA transpose degenerates to 4-byte packets and takes ~12us --
        measured; it throttled the whole mid-kernel in v1/v2.)"""
        colmax = const.tile([128, 8], F32, tag=f"cm{h}", name=f"cm{h}")
        for i in range(8):
            ps = pool.tile([128, T], F32, tag="av" if pool is ps_av else "big",
                           name="psm")
            for n in range(2):
                nsl = slice(n * 512, (n + 1) * 512)
                nc.tensor.matmul(
                    ps[:, nsl],
                    lhsT=qm[h][0:64, i * 128:(i + 1) * 128],
                    rhs=km[h][0:64, nsl],
                    start=True, stop=True,
                )
            nc.vector.reduce_max(colmax[:, i:i + 1], ps[:], axis=AX.X,
                                 negate=True)
            yield
        pst = pool.tile([8, 128], F32, tag="av" if pool is ps_av else "big",
                        name=f"pst{h}")
        nc.tensor.transpose(pst[:], colmax[:], ident[:])
        qmx = const.tile([8, 128], F16, tag=f"qmx{h}", name=f"qmx{h}")
        nc.scalar.copy(qmx[:], pst[:])
        sc = dramp.tile([8, 128], F16, tag=f"sc{h}", name=f"sc{h}")
        nc.sync.dma_start(sc[:], qmx[:])
        nc.sync.dma_start(qm[h][64:65, :], sc[:].rearrange("c p -> (c p)"))
        while True:
            yield

    def vp_pass():
        for t in range(8):
            ps = ps_big.tile([128, DX], F32, tag="big", name="psv")
            for e in range(4):
                nc.tensor.matmul(
                    ps[:],
                    lhsT=kvh_in[e][:, t * 128:(t + 1) * 128],
                    rhs=wv[e][:],
                    start=(e == 0), stop=(e == 3),
                )
            nc.scalar.copy(vpx[t][:], ps[:])
            for h4 in range(NH):
                c = h4 * (D + 1) + D
                nc.vector.memset(vpx[t][:, c:c + 1], 1.0)

    def norm_head(h, oex):
        """Normalize head h's AV result: yun = oex rows 0:64 (DVE evac),
        s = oex row 64; 1/s = exp(-ln s) on ACT (fp16 out, 5e-4 rel --
        well under the bf16 A-tile error).  The 64-partition broadcast is
        a rank-1 PE matmul ones_col^T @ recip into PSUM (a DMA bounce
        through DRAM costs 3-4us of latency here; the matmul ~0.5us).
        yh half = yun * recb (DVE, fp16 out -- values are V-scale after
        the normalize, fp16-safe)."""
        g, half = divmod(h, 2)
        # Heads 0-2 skip the [64,T] PSUM evacuation entirely: the final
        # multiply reads oex rows 0:64 straight from PSUM (gpsimd cannot
        # access PSUM, and DVE is the per-head pacer -- a separate copy
        # would cost another 1.2us there).  The oex slot is then held
        # until the mul, which the next head's lagged AV absorbs.  Head 3
        # keeps the evac on the idle-at-tail DVE so its mul can pair with
        # the PSUM-resident PE-broadcast recb.
        yun = oex[0:64, :]
        if h == 3:
            yun3 = const.tile([64, T], F32, tag="yun3", name="yun3")
            nc.vector.tensor_copy(yun3[:], oex[0:64, :])
            yun = yun3[:]
        lns = const.tile([1, T], F32, tag=f"ln{h}", name=f"ln{h}")
        nc.scalar.activation(lns[:], oex[64:65, :], AF.Ln)
        if h == 3:
            # Tail head: broadcast via a K=1 matmul into PSUM -- PE is
            # idle here and this cuts the DRAM round-trip off the final
            # serial chain.  (Mid-kernel this stalls the PE FIFO, so the
            # other heads use the DMA bounce instead.)  bf16 recip: f32
            # matmuls are 4 passes/col (~4us measured for the pair); bf16
            # keeps f32 RANGE (1/s spans e^+-15) and its 2^-9 rounding
            # only touches head 3's quarter of the output (~0.005 abs).
            recip_b = const.tile([1, T], BF16, tag="rcpb", name="rcpb")
            nc.scalar.activation(recip_b[:], lns[:], AF.Exp, scale=-1.0)
            recb = ps_big.tile([64, T], F32, tag="big", name=f"rcb{h}")
            for n in range(2):
                nsl = slice(n * 512, (n + 1) * 512)
                nc.tensor.matmul(recb[:, nsl], lhsT=ones_col[:],
                                 rhs=recip_b[:, nsl], start=True, stop=True)
            nc.vector.tensor_mul(yh[g][half * 64:(half + 1) * 64, :],
                                 yun, recb[:])
            return
        # f32: 1/s spans [e^-15, e^15] (the bias-row error window), which
        # overflows fp16; the DMA broadcast doesn't care about dtype.
        recip = const.tile([1, T], F32, tag=f"rcp{h}", name=f"rcp{h}")
        nc.scalar.activation(recip[:], lns[:], AF.Exp, scale=-1.0)
        rrow = dramp.tile([1, T], F32, tag=f"rr{h}", name=f"rr{h}")
        nc.sync.dma_start(rrow[:], recip[:])
        recb = const.tile([64, T], F32, tag=f"rcb{h}", name=f"rcb{h}")
        rap = rrow[:]
        bcast = bass.AP(
            tensor=rap.tensor, offset=rap.offset,
            ap=[[0, 64]] + [list(d) for d in rap.ap[1:]],
        )
        nc.sync.dma_start(recb[:], bcast)
        nc.vector.tensor_mul(yh[g][half * 64:(half + 1) * 64, :], yun,
                             recb[:])

    def wo_block(g, i, copy_eng=None):
        pso = ps_big.tile([128, E], F32, tag="big", name="pso")
        nc.tensor.matmul(
            pso[:],
            lhsT=yh[g][:, i * 128:(i + 1) * 128],
            rhs=wo[g][:],
            start=True, stop=True,
        )
        osb = outp.tile([128, E], F32, tag="ot", name=f"ot{g}_{i}")
        if copy_eng == "scalar":
            nc.scalar.copy(osb[:], pso[:])
        else:
            nc.vector.tensor_copy(osb[:], pso[:])
        nc.sync.dma_start(dram[f"out{g}"][i * 128:(i + 1) * 128, :], osb[:])

    def mainpass(h, weave=None, wo_weave=None, norm_prev=None,
                 split_cross=False):
        """S^T - m = cross + main(bias row), exp (ACT, bf16), AV into oex.
        The AV matmuls lag one j-block behind S^T/exp so the PE FIFO is
        never parked on an AV that transitively waits on the previous
        head's normalize chain (oex slot WAR) or on ACT's exp cadence.
        `weave`: generator advanced once per step (maxpass of h+2).
        `wo_weave`: (g, [(step, i)...]) wo blocks emitted at given steps.
        `norm_prev`: head whose normalize chain rides this pass's start.
        `split_cross`: emit the cross matmul as two K=64 halves (exactly
        the same PSUM accumulation) -- pure PE-density padding so a pass
        with nothing woven in still keeps the HAM clock gate at 8/8."""
        oex = ps_av.tile([65, T], F32, tag="av", name="oex")
        ats = {}
        for j in range(9):
            if j < 8:
                jsl = slice(j * 128, (j + 1) * 128)
                ps = ps_big.tile([128, T], F32, tag="big", name="psb")
                for n in range(2):
                    nsl = slice(n * 512, (n + 1) * 512)
                    if split_cross:
                        nc.tensor.matmul(
                            ps[:, nsl], lhsT=kc[h][0:64, jsl],
                            rhs=qc[h][0:64, nsl], start=True, stop=False,
                        )
                        nc.tensor.matmul(
                            ps[:, nsl], lhsT=kc[h][64:128, jsl],
                            rhs=qc[h][64:128, nsl], start=False, stop=False,
                        )
                    else:
                        nc.tensor.matmul(
                            ps[:, nsl], lhsT=kc[h][:, jsl], rhs=qc[h][:, nsl],
                            start=True, stop=False,
                        )
                    nc.tensor.matmul(
                        ps[:, nsl], lhsT=km[h][:, jsl], rhs=qm[h][:, nsl],
                        start=False, stop=True,
                    )
                at = atp.tile([128, T], BF16, tag="at", name="at")
                nc.scalar.activation(at[:], ps[:], AF.Exp)
                ats[j] = at
            if j == 1 and norm_prev is not None:
                norm_head(*norm_prev)
            if j >= 1:
                ja = j - 1
                at = ats.pop(ja)
                for n in range(2):
                    nsl = slice(n * 512, (n + 1) * 512)
                    nc.tensor.matmul(
                        oex[:, nsl],
                        lhsT=vpx[ja][:, h * (D + 1):(h + 1) * (D + 1)],
                        rhs=at[:, nsl],
                        start=(ja == 0), stop=(ja == 7),
                    )
            if weave is not None:
                # Mildly front-loaded (2 blocks at steps 0-1, then 1 per
                # step; transpose/cast/DMA chain at step 6) so the NEXT
                # head's bias row lands before the pass boundary without
                # over-subscribing the psum ring early in the pass.
                next(weave, None)
                if j < 2:
                    next(weave, None)
            if wo_weave is not None:
                g, blocks = wo_weave
                for step, i in blocks:
                    if step == j:
                        wo_block(g, i, copy_eng="scalar" if i % 2 else None)
        return oex

    # ---- emission schedule ----
    mx = [maxpass_gen(h, ps_big) for h in range(NH)]

    proj_pass(wqh, wql, qh_in, ql_in, qm, qc, 64, 0)             # q heads 0,1
    proj_pass(wkh, wkl, kvh_in, kvl_in, km, kc, 0, 0)            # k heads 0,1
    for h in range(NH):
        nc.vector.memset(km[h][64:65, :], 1.0)
    # maxpass(0) spreads over BOTH m=1 projection streams (1 i-block per
    # 6 proj MMs) so PE stays denser than the DVE reduce cadence; the
    # other heads' maxpasses ride the mainpasses one head ahead.
    proj_pass(wqh, wql, qh_in, ql_in, qm, qc, 64, 1,
              weave=mx[0], weave_every=6)                        # q heads 2,3
    proj_pass(wkh, wkl, kvh_in, kvl_in, km, kc, 0, 1,
              weave=mx[0], weave_every=6)                        # k heads 2,3
    next(mx[0], None)  # bias-row transpose/cast/DMA chain
    vp_pass()

    oex0 = mainpass(0, weave=mx[1])
    oex1 = mainpass(1, weave=mx[2], norm_prev=(0, oex0))
    oex2 = mainpass(2, weave=mx[3],
                    wo_weave=(0, [(j, j - 5) for j in range(5, 9)]),
                    norm_prev=(1, oex1))
    oex3 = mainpass(3, wo_weave=(0, [(j, j + 4) for j in range(0, 4)]),
                    norm_prev=(2, oex2))
    norm_head(3, oex3)
    for i in range(8):
        wo_block(1, i, copy_eng="scalar" if i % 2 else None)


class _Bacc(bacc.Bacc):
    """Bacc whose activation-table chooser can only pick the combined
    natural_log_exp_and_others set for Exp/Ln: the exp-only sets are
    blanked (entries kept so act_func_set_id indices stay aligned with
    act_info.json), which removes the per-head Exp<->Ln table swap
    (~2.7us each)."""

    def insert_act_table_loads(self):
        import bass_rust as _bass_rust
        from concourse.hw_specs import get_activation_tables
        has_activation = any(
            isinstance(i, mybir.InstActivation)
            for b in self.main_func.blocks
            for i in b.instructions
        )
        if not has_activation:
            return
        tables = []
        for name, fns in get_activation_tables(self.m.arch).items():
            if name in ("exp_and_others", "exp_and_friends"):
                fns = set()
            tables.append((name, fns))
        _bass_rust.insert_act_table_loads(self, tables)


def build_program():
    # Bacc (not raw Bass): its compile() splits multi-sem matmul waits onto
    # ldweights (TRN2 allows 1 wait/instruction) and lowers extended-ISA.
    nc = _Bacc("TRN2", target_bir_lowering=False, debug=False)
    dp = nc.declare_dram_parameter
    dram = {}
    for name in ("qh", "ql", "kvh", "kvl"):
        dram[name] = dp(name, [E, T], F16, isOutput=False)
    for name in ("wqh", "wql", "wkh", "wkl"):
        dram[name] = dp(name, [E, NH * D], F16, isOutput=False)
    dram["wv"] = dp("wv", [E, DX], F16, isOutput=False)
    dram["wo"] = dp("wo", [NH * D, E], F16, isOutput=False)
    dram["ident"] = dp("ident", [128, 128], F32, isOutput=False)
    dram["out0"] = dp("out0", [T, E], F32, isOutput=True)
    dram["out1"] = dp("out1", [T, E], F32, isOutput=True)
    with ExitStack() as ctx:
        tc = ctx.enter_context(tile.TileContext(nc))
        _emit(ctx, tc, dram)
    nc.finalize()  # Bacc.finalize runs compile() then freezes
    return nc


_PROGRAM = None


def _get_program():
    global _PROGRAM
    if _PROGRAM is None:
        _PROGRAM = build_program()
    return _PROGRAM


def _split16(x):
    h = x.astype(np.float16)
    l = (x - h.astype(np.float32)).astype(np.float16)
    return h, l


def make_in_maps(q, kv, Wq, Wk, Wv, Wo):
    in_maps = []
    for c in range(N_CORES):
        b, g = divmod(c, 2)
        heads = [g * NH + j for j in range(NH)]
        idx_q = [d * H + h for h in heads for d in range(D)]
        idx_k = [((D - d) % D) * H + h for h in heads for d in range(D)]
        qTh, qTl = _split16(np.ascontiguousarray(q[b].T))
        kvTh, kvTl = _split16(np.ascontiguousarray(kv[b].T))
        wq_h, wq_l = _split16(Wq[:, idx_q] * np.float32(SCALE))
        wk_h, wk_l = _split16(Wk[:, idx_k])
        wv_c = np.zeros((E, DX), np.float16)
        for j, h in enumerate(heads):
            wv_c[:, j * (D + 1):j * (D + 1) + D] = \
                Wv[:, [d * H + h for d in range(D)]].astype(np.float16)
        in_maps.append({
            "qh": qTh, "ql": qTl, "kvh": kvTh, "kvl": kvTl,
            "wqh": wq_h, "wql": wq_l, "wkh": wk_h, "wkl": wk_l,
            "wv": wv_c,
            "wo": Wo[g * NH * D:(g + 1) * NH * D, :].astype(np.float16),
            "ident": np.eye(128, dtype=np.float32),
        })
    return in_maps


def kernel(**inputs):
    global LAST_EXEC_NS
    q = np.asarray(inputs["q"], dtype=np.float32)
    kv = np.asarray(inputs["kv"], dtype=np.float32)
    Wq = np.asarray(inputs["Wq"], dtype=np.float32)
    Wk = np.asarray(inputs["Wk"], dtype=np.float32)
    Wv = np.asarray(inputs["Wv"], dtype=np.float32)
    Wo = np.asarray(inputs["Wo"], dtype=np.float32)
    B = q.shape[0]

    nc = _get_program()
    in_maps = make_in_maps(q, kv, Wq, Wk, Wv, Wo)
    res = run_bass_kernel_spmd(nc, in_maps, list(range(N_CORES)), trace=TRACE)
    LAST_EXEC_NS = res.exec_time_ns

    out = np.empty((B, T, E), np.float32)
    for b in range(B):
        r0, r1 = res.results[2 * b], res.results[2 * b + 1]
        out[b] = (r0["out0"] + r0["out1"]) + (r1["out0"] + r1["out1"])
    return out


# revision 46
# speedup vs baseline: 1.0706x; 1.0296x over previous
"""Trainium2 Bass kernel for MultiHeadFrequencyCrossAttention.

Math note: the reference computes, per (batch, head) slice,
    energy = ifft2( fft2(Q) @ fft2(K)^T * dk ).real
Because the DFT matrix F satisfies F @ F^T = n * P (P = index-negation
permutation), this collapses EXACTLY to
    energy = dk * D * Q @ K~^T        with K~[j, d] = K[j, (-d) mod D]
i.e. plain attention with K's head-dim index flipped (mod D) and an extra
scale of dk * D = 512.  No FFTs are needed; the flip and scale are folded
into host-side slices of the Wk / Wq projection weights.

Sharding: 8 cores = 4 batches x 2 head-groups (4 heads each).  Each core
gets q[b]^T, kv[b]^T (pre-transposed on host so the contraction dim lands
on SBUF partitions) plus its slice of the projection weights, computes
attention for its 4 heads, and applies its slice of Wo.  The host sums the
four partial Wo products per batch (two cores x two wo subgroups).

Precision scheme (PE fp32 matmuls are 4 cyc/row; fp16 is 1 cyc/row):
every value on the logit path is split hi/lo into two fp16 parts
(x = xh + xl, products of fp16 are exact in the fp32 PSUM accumulator), so
  x @ y ~= xh@yh + (xh@yl + xl@yh)     [~22-bit mantissa, err ~1e-6 rel]
One extra all-ones row in the stationary K operand times a "-rowmax" row
in the moving Q operand injects the softmax max-subtraction bias directly
into the S^T matmul.  The row max itself comes from a separate hi-only
fp16 pass (error ~ +-15 absolute on ~25000-scale logits, well inside the
exp() range window since A tiles are bf16).  A/V/output paths are plain
16-bit (error there stays relative, ~2e-3, no sharp-softmax blowup).

Engine assignment (v2 -- rebalanced so ACT only does exp/ln work and
gpsimd is not used at all, which avoids its library swaps):
  PE   : all matmuls (warmup, projections, max pass, S^T, AV, Wo)
  ACT  : A = exp(S^T - m) psum->sbuf, per-head ln(s) / exp(-ln s), and
         the proj hi evacuations (proj phase only, PE-bound there)
  DVE  : reduce_max (negated, fp16 out), proj lo subs, yun evacuation,
         y * (1/s) normalize (writes fp16 yh directly), wo psum->sbuf
  DMA  : inputs (chunked), colmax fp16 bounce straight into qm bias row,
         1/s row broadcast to 64 partitions (stride-0 descriptor),
         per-i-block output stores (separate out0/out1, host adds)

Emission order software-pipelines everything: maxpass(0)/(1) matmuls are
woven into the m=1 projection streams, maxpass(2)/(3) into mainpass(0)/(1)
j-blocks, wo(0) into mainpass(2), so PE never idles long enough for the
HAM clock gate to re-throttle (the v1 kernel spent 110us at half clock).
"""

import numpy as np
from contextlib import ExitStack

import concourse.bass as bass
import concourse.tile as tile
from concourse import bacc, mybir
from concourse.bass_utils import run_bass_kernel_spmd

F32 = mybir.dt.float32
F16 = mybir.dt.float16
BF16 = mybir.dt.bfloat16
AX = mybir.AxisListType
AF = mybir.ActivationFunctionType

T = 1024          # sequence length
E = 512           # embed dim
H = 8             # total heads
D = E // H        # head dim = 64
NH = 4            # heads per core
DX = NH * (D + 1) # vp columns incl. ones = 260
N_CORES = 8
SCALE = float(D) * float(D) ** 0.5  # dk * D = 512.0

TRACE = False          # set by test harness; adds NTFF profiling
LAST_EXEC_NS = None


def _emit(ctx, tc, dram):
    nc = tc.nc
    const = ctx.enter_context(tc.tile_pool(name="const", bufs=1))
    ps_big = ctx.enter_context(tc.tile_pool(name="ps_big", bufs=3, space="PSUM"))
    ps_av = ctx.enter_context(tc.tile_pool(name="ps_av", bufs=1, space="PSUM"))
    atp = ctx.enter_context(tc.tile_pool(name="atp", bufs=4))
    outp = ctx.enter_context(tc.tile_pool(name="outp", bufs=4))
    dramp = ctx.enter_context(tc.tile_pool(name="dramp", bufs=1, space="DRAM"))

    # ---- input loads (all fp16 on the wire) ----
    # The four big [E, T] tensors are split into T-halves so the first
    # projection block can start after ~1.5 MB instead of ~2.5 MB.
    tiles3 = {}
    def declare(name, cols):
        t3 = const.tile([128, 4, cols], F16, tag=name, name=name)
        tiles3[name] = t3
        return [t3[:, e, :] for e in range(4)]

    def load_half(name, half, eng=None):
        t3 = tiles3[name]
        cols = t3.shape[2]
        h0, h1 = half * cols // 2, (half + 1) * cols // 2
        (eng or nc.sync).dma_start(
            t3[:, :, h0:h1],
            dram[name][:, h0:h1].rearrange("(c p) t -> p c t", p=128),
        )

    def load_full(name, eng=None):
        t3 = tiles3[name]
        (eng or nc.sync).dma_start(
            t3[:], dram[name][:].rearrange("(c p) t -> p c t", p=128)
        )

    wqh = declare("wqh", NH * D)
    wql = declare("wql", NH * D)
    qh_in = declare("qh", T)
    ql_in = declare("ql", T)
    wkh = declare("wkh", NH * D)
    wkl = declare("wkl", NH * D)
    kvh_in = declare("kvh", T)
    kvl_in = declare("kvl", T)
    wv = declare("wv", DX)
    wo3 = const.tile([128, 2, E], F16, tag="wo", name="wo")
    wo = [wo3[:, g, :] for g in range(2)]
    ident = const.tile([128, 128], F32, tag="ident", name="ident")
    nc.sync.dma_start(ident[:], dram["ident"][:])

    # One queue, strict priority order (two queues just split the HBM
    # bandwidth and delay the critical q-proj operands -- measured).
    load_full("wqh"); load_full("wql")
    load_half("ql", 0); load_half("qh", 0)
    load_half("ql", 1); load_half("qh", 1)
    load_full("wkh"); load_full("wkl")
    load_half("kvl", 0); load_half("kvh", 0)
    load_half("kvl", 1); load_half("kvh", 1)
    load_full("wv")
    nc.sync.dma_start(wo3[:], dram["wo"][:].rearrange("(g p) t -> p g t", p=128))

    # PE warm-up: dummy matmuls fill the input-DMA window so the HAM clock
    # gate is already at 8/8 (2.4 GHz) when the projections start.  18 MMs
    # x ~430ns cold spans the ~8us until the first projection operands land.
    wrm = const.tile([128, 512], F16, tag="wrm", name="wrm")
    nc.vector.memset(wrm[:], 0.0)
    ones_col = const.tile([1, 64], BF16, tag="ones_col", name="ones_col")
    nc.vector.memset(ones_col[:], 1.0)
    # Dummy Exp/Ln so the natural_log_exp table set loads inside the input
    # DMA window instead of stalling ACT at mainpass(0)'s first exp.
    dtab = const.tile([1, 2], F32, tag="dtab", name="dtab")
    nc.scalar.activation(dtab[:], wrm[0:1, 0:2], AF.Exp)
    nc.scalar.activation(dtab[:], wrm[0:1, 0:2], AF.Ln)
    for w in range(26):
        pw = ps_big.tile([128, 512], F32, tag="big", name="psw")
        nc.tensor.matmul(pw[:], lhsT=wrm[:, 0:128], rhs=wrm[:],
                         start=True, stop=True)

    # ---- per-head SBUF tensors ----
    qm = [const.tile([65, T], F16, tag=f"qm{h}", name=f"qm{h}") for h in range(NH)]
    km = [const.tile([65, T], F16, tag=f"km{h}", name=f"km{h}") for h in range(NH)]
    qc = [const.tile([128, T], F16, tag=f"qc{h}", name=f"qc{h}") for h in range(NH)]
    kc = [const.tile([128, T], F16, tag=f"kc{h}", name=f"kc{h}") for h in range(NH)]
    vpx = [const.tile([128, DX], BF16, tag=f"vpx{t}", name=f"vpx{t}")
           for t in range(8)]
    yh = [const.tile([128, T], F16, tag=f"yh{g}", name=f"yh{g}") for g in range(2)]

    # ---- emission helpers; `weave` generators let one pass's matmuls be
    # interleaved into another pass's PE stream ----

    def proj_pass(wh, wl, xh, xl, dm, dc, hi_row, m, weave=None,
                  weave_every=3):
        """hi/lo projection for head pair m: 24 matmuls + evacuations.
        dm gets the fp16 hi part (rows 0:64); dc the [lo;hi] stack."""
        msl = slice(m * 128, (m + 1) * 128)
        ps = ps_big.tile([128, T], F32, tag="big", name="psb")
        n_mm = 0
        for n in range(2):
            nsl = slice(n * 512, (n + 1) * 512)
            mms = (
                [(wh[e], xl[e]) for e in range(4)]
                + [(wl[e], xh[e]) for e in range(4)]
                + [(wh[e], xh[e]) for e in range(4)]
            )
            for i_mm, (lw, rx) in enumerate(mms):
                nc.tensor.matmul(
                    ps[:, nsl],
                    lhsT=lw[:, msl],
                    rhs=rx[:, nsl],
                    start=(i_mm == 0), stop=(i_mm == len(mms) - 1),
                )
                n_mm += 1
                if weave is not None and n_mm % weave_every == 0:
                    next(weave, None)
        for hh in range(2):
            h = 2 * m + hh
            psl = slice(hh * 64, hh * 64 + 64)
            lo_row = 64 - hi_row
            # hi part (fp16 cast) -> K=65 "main" tile rows 0:64 (ACT --
            # PSUM source, so gpsimd can't take it)
            nc.scalar.copy(dm[h][0:64, :], ps[psl, :])
            # hi copy into the cross tile: SBUF->SBUF fp16, so it rides
            # the otherwise-idle gpsimd (ACT was evac-saturated here)
            nc.gpsimd.tensor_copy(dc[h][hi_row:hi_row + 64, :],
                                  dm[h][0:64, :])
            # lo part = ps - hi  (DVE)
            nc.vector.tensor_sub(dc[h][lo_row:lo_row + 64, :], ps[psl, :],
                                 dm[h][0:64, :])

    def maxpass_gen(h, pool):
        """Generator: one next() emits one i-block (2 MMs + DVE reduce).
        `pool` is the PSUM pool the S tiles rotate through: ps_av during
        the projection phases (it is idle there, which keeps the weave
        from stalling the projection's long-lived ps_big accumulator),
        ps_big when woven into a mainpass.

        The (-max) column tile is turned into the fp16 bias row of qm[h]
        via PE transpose -> DVE cast -> one coalesced DMA.  (A strided
        DMA transpose degenerates to 4-byte packets and takes ~12us --
        measured; it throttled the whole mid-kernel in v1/v2.)"""
        colmax = const.tile([128, 8], F32, tag=f"cm{h}", name=f"cm{h}")
        for i in range(8):
            ps = pool.tile([128, T], F32, tag="av" if pool is ps_av else "big",
                           name="psm")
            for n in range(2):
                nsl = slice(n * 512, (n + 1) * 512)
                nc.tensor.matmul(
                    ps[:, nsl],
                    lhsT=qm[h][0:64, i * 128:(i + 1) * 128],
                    rhs=km[h][0:64, nsl],
                    start=True, stop=True,
                )
            nc.vector.reduce_max(colmax[:, i:i + 1], ps[:], axis=AX.X,
                                 negate=True)
            yield
        pst = pool.tile([8, 128], F32, tag="av" if pool is ps_av else "big",
                        name=f"pst{h}")
        nc.tensor.transpose(pst[:], colmax[:], ident[:])
        qmx = const.tile([8, 128], F16, tag=f"qmx{h}", name=f"qmx{h}")
        nc.scalar.copy(qmx[:], pst[:])
        sc = dramp.tile([8, 128], F16, tag=f"sc{h}", name=f"sc{h}")
        nc.sync.dma_start(sc[:], qmx[:])
        nc.sync.dma_start(qm[h][64:65, :], sc[:].rearrange("c p -> (c p)"))
        while True:
            yield

    def vp_pass():
        for t in range(8):
            ps = ps_big.tile([128, DX], F32, tag="big", name="psv")
            for e in range(4):
                nc.tensor.matmul(
                    ps[:],
                    lhsT=kvh_in[e][:, t * 128:(t + 1) * 128],
                    rhs=wv[e][:],
                    start=(e == 0), stop=(e == 3),
                )
            nc.scalar.copy(vpx[t][:], ps[:])
            for h4 in range(NH):
                c = h4 * (D + 1) + D
                nc.vector.memset(vpx[t][:, c:c + 1], 1.0)

    def norm_head(h, oex):
        """Normalize head h's AV result: yun = oex rows 0:64 (DVE evac),
        s = oex row 64; 1/s = exp(-ln s) on ACT (fp16 out, 5e-4 rel --
        well under the bf16 A-tile error).  The 64-partition broadcast is
        a rank-1 PE matmul ones_col^T @ recip into PSUM (a DMA bounce
        through DRAM costs 3-4us of latency here; the matmul ~0.5us).
        yh half = yun * recb (DVE, fp16 out -- values are V-scale after
        the normalize, fp16-safe)."""
        g, half = divmod(h, 2)
        # Heads 0-2 skip the [64,T] PSUM evacuation entirely: the final
        # multiply reads oex rows 0:64 straight from PSUM (gpsimd cannot
        # access PSUM, and DVE is the per-head pacer -- a separate copy
        # would cost another 1.2us there).  The oex slot is then held
        # until the mul, which the next head's lagged AV absorbs.  Head 3
        # keeps the evac on the idle-at-tail DVE so its mul can pair with
        # the PSUM-resident PE-broadcast recb.
        yun = oex[0:64, :]
        if h == 3:
            yun3 = const.tile([64, T], F32, tag="yun3", name="yun3")
            nc.vector.tensor_copy(yun3[:], oex[0:64, :])
            yun = yun3[:]
        lns = const.tile([1, T], F32, tag=f"ln{h}", name=f"ln{h}")
        nc.scalar.activation(lns[:], oex[64:65, :], AF.Ln)
        if h == 3:
            # Tail head: broadcast via a K=1 matmul into PSUM -- PE is
            # idle here and this cuts the DRAM round-trip off the final
            # serial chain.  (Mid-kernel this stalls the PE FIFO, so the
            # other heads use the DMA bounce instead.)  bf16 recip: f32
            # matmuls are 4 passes/col (~4us measured for the pair); bf16
            # keeps f32 RANGE (1/s spans e^+-15) and its 2^-9 rounding
            # only touches head 3's quarter of the output (~0.005 abs).
            recip_b = const.tile([1, T], BF16, tag="rcpb", name="rcpb")
            nc.scalar.activation(recip_b[:], lns[:], AF.Exp, scale=-1.0)
            recb = ps_big.tile([64, T], F32, tag="big", name=f"rcb{h}")
            for n in range(2):
                nsl = slice(n * 512, (n + 1) * 512)
                nc.tensor.matmul(recb[:, nsl], lhsT=ones_col[:],
                                 rhs=recip_b[:, nsl], start=True, stop=True)
            nc.vector.tensor_mul(yh[g][half * 64:(half + 1) * 64, :],
                                 yun, recb[:])
            return
        # f32: 1/s spans [e^-15, e^15] (the bias-row error window), which
        # overflows fp16; the DMA broadcast doesn't care about dtype.
        recip = const.tile([1, T], F32, tag=f"rcp{h}", name=f"rcp{h}")
        nc.scalar.activation(recip[:], lns[:], AF.Exp, scale=-1.0)
        rrow = dramp.tile([1, T], F32, tag=f"rr{h}", name=f"rr{h}")
        nc.sync.dma_start(rrow[:], recip[:])
        recb = const.tile([64, T], F32, tag=f"rcb{h}", name=f"rcb{h}")
        rap = rrow[:]
        bcast = bass.AP(
            tensor=rap.tensor, offset=rap.offset,
            ap=[[0, 64]] + [list(d) for d in rap.ap[1:]],
        )
        nc.sync.dma_start(recb[:], bcast)
        nc.vector.tensor_mul(yh[g][half * 64:(half + 1) * 64, :], yun,
                             recb[:])

    def wo_block(g, i, copy_eng=None):
        pso = ps_big.tile([128, E], F32, tag="big", name="pso")
        nc.tensor.matmul(
            pso[:],
            lhsT=yh[g][:, i * 128:(i + 1) * 128],
            rhs=wo[g][:],
            start=True, stop=True,
        )
        osb = outp.tile([128, E], F32, tag="ot", name=f"ot{g}_{i}")
        if copy_eng == "scalar":
            nc.scalar.copy(osb[:], pso[:])
        else:
            nc.vector.tensor_copy(osb[:], pso[:])
        nc.sync.dma_start(dram[f"out{g}"][i * 128:(i + 1) * 128, :], osb[:])

    def mainpass(h, weave=None, wo_weave=None, norm_prev=None,
                 split_cross=False):
        """S^T - m = cross + main(bias row), exp (ACT, bf16), AV into oex.
        The AV matmuls lag one j-block behind S^T/exp so the PE FIFO is
        never parked on an AV that transitively waits on the previous
        head's normalize chain (oex slot WAR) or on ACT's exp cadence.
        `weave`: generator advanced once per step (maxpass of h+2).
        `wo_weave`: (g, [(step, i)...]) wo blocks emitted at given steps.
        `norm_prev`: head whose normalize chain rides this pass's start.
        `split_cross`: emit the cross matmul as two K=64 halves (exactly
        the same PSUM accumulation) -- pure PE-density padding so a pass
        with nothing woven in still keeps the HAM clock gate at 8/8."""
        oex = ps_av.tile([65, T], F32, tag="av", name="oex")
        ats = {}
        for j in range(9):
            if j < 8:
                jsl = slice(j * 128, (j + 1) * 128)
                ps = ps_big.tile([128, T], F32, tag="big", name="psb")
                for n in range(2):
                    nsl = slice(n * 512, (n + 1) * 512)
                    if split_cross:
                        nc.tensor.matmul(
                            ps[:, nsl], lhsT=kc[h][0:64, jsl],
                            rhs=qc[h][0:64, nsl], start=True, stop=False,
                        )
                        nc.tensor.matmul(
                            ps[:, nsl], lhsT=kc[h][64:128, jsl],
                            rhs=qc[h][64:128, nsl], start=False, stop=False,
                        )
                    else:
                        nc.tensor.matmul(
                            ps[:, nsl], lhsT=kc[h][:, jsl], rhs=qc[h][:, nsl],
                            start=True, stop=False,
                        )
                    nc.tensor.matmul(
                        ps[:, nsl], lhsT=km[h][:, jsl], rhs=qm[h][:, nsl],
                        start=False, stop=True,
                    )
                at = atp.tile([128, T], BF16, tag="at", name="at")
                nc.scalar.activation(at[:], ps[:], AF.Exp)
                ats[j] = at
            if j == 1 and norm_prev is not None:
                norm_head(*norm_prev)
            if j >= 1:
                ja = j - 1
                at = ats.pop(ja)
                for n in range(2):
                    nsl = slice(n * 512, (n + 1) * 512)
                    nc.tensor.matmul(
                        oex[:, nsl],
                        lhsT=vpx[ja][:, h * (D + 1):(h + 1) * (D + 1)],
                        rhs=at[:, nsl],
                        start=(ja == 0), stop=(ja == 7),
                    )
            if weave is not None:
                # Mildly front-loaded (2 blocks at steps 0-1, then 1 per
                # step; transpose/cast/DMA chain at step 6) so the NEXT
                # head's bias row lands before the pass boundary without
                # over-subscribing the psum ring early in the pass.
                next(weave, None)
                if j < 2:
                    next(weave, None)
            if wo_weave is not None:
                g, blocks = wo_weave
                for step, i in blocks:
                    if step == j:
                        wo_block(g, i, copy_eng="scalar" if i % 2 else None)
        return oex

    # ---- emission schedule ----
    mx = [maxpass_gen(h, ps_big) for h in range(NH)]

    proj_pass(wqh, wql, qh_in, ql_in, qm, qc, 64, 0)             # q heads 0,1
    proj_pass(wkh, wkl, kvh_in, kvl_in, km, kc, 0, 0)            # k heads 0,1
    for h in range(NH):
        nc.vector.memset(km[h][64:65, :], 1.0)
    # maxpass(0) spreads over BOTH m=1 projection streams (1 i-block per
    # 6 proj MMs) so PE stays denser than the DVE reduce cadence; the
    # other heads' maxpasses ride the mainpasses one head ahead.
    proj_pass(wqh, wql, qh_in, ql_in, qm, qc, 64, 1,
              weave=mx[0], weave_every=6)                        # q heads 2,3
    proj_pass(wkh, wkl, kvh_in, kvl_in, km, kc, 0, 1,
              weave=mx[0], weave_every=6)                        # k heads 2,3
    next(mx[0], None)  # bias-row transpose/cast/DMA chain
    vp_pass()

    oex0 = mainpass(0, weave=mx[1])
    oex1 = mainpass(1, weave=mx[2], norm_prev=(0, oex0))
    oex2 = mainpass(2, weave=mx[3],
                    wo_weave=(0, [(j, j - 5) for j in range(5, 9)]),
                    norm_prev=(1, oex1))
    oex3 = mainpass(3, wo_weave=(0, [(j, j + 4) for j in range(0, 4)]),
                    norm_prev=(2, oex2))
    norm_head(3, oex3)
    for i in range(8):
        wo_block(1, i, copy_eng="scalar" if i % 2 else None)


class _Bacc(bacc.Bacc):
    """Bacc whose activation-table chooser can only pick the combined
    natural_log_exp_and_others set for Exp/Ln: the exp-only sets are
    blanked (entries kept so act_func_set_id indices stay aligned with
    act_info.json), which removes the per-head Exp<->Ln table swap
    (~2.7us each)."""

    def insert_act_table_loads(self):
        import bass_rust as _bass_rust
        from concourse.hw_specs import get_activation_tables
        has_activation = any(
            isinstance(i, mybir.InstActivation)
            for b in self.main_func.blocks
            for i in b.instructions
        )
        if not has_activation:
            return
        tables = []
        for name, fns in get_activation_tables(self.m.arch).items():
            if name in ("exp_and_others", "exp_and_friends"):
                fns = set()
            tables.append((name, fns))
        _bass_rust.insert_act_table_loads(self, tables)


def build_program():
    # Bacc (not raw Bass): its compile() splits multi-sem matmul waits onto
    # ldweights (TRN2 allows 1 wait/instruction) and lowers extended-ISA.
    nc = _Bacc("TRN2", target_bir_lowering=False, debug=False)
    dp = nc.declare_dram_parameter
    dram = {}
    for name in ("qh", "ql", "kvh", "kvl"):
        dram[name] = dp(name, [E, T], F16, isOutput=False)
    for name in ("wqh", "wql", "wkh", "wkl"):
        dram[name] = dp(name, [E, NH * D], F16, isOutput=False)
    dram["wv"] = dp("wv", [E, DX], F16, isOutput=False)
    dram["wo"] = dp("wo", [NH * D, E], F16, isOutput=False)
    dram["ident"] = dp("ident", [128, 128], F32, isOutput=False)
    dram["out0"] = dp("out0", [T, E], F32, isOutput=True)
    dram["out1"] = dp("out1", [T, E], F32, isOutput=True)
    with ExitStack() as ctx:
        tc = ctx.enter_context(tile.TileContext(nc))
        _emit(ctx, tc, dram)
    nc.finalize()  # Bacc.finalize runs compile() then freezes
    return nc


_PROGRAM = None


def _get_program():
    global _PROGRAM
    if _PROGRAM is None:
        _PROGRAM = build_program()
    return _PROGRAM


def _split16(x):
    h = x.astype(np.float16)
    l = (x - h.astype(np.float32)).astype(np.float16)
    return h, l


def make_in_maps(q, kv, Wq, Wk, Wv, Wo):
    in_maps = []
    for c in range(N_CORES):
        b, g = divmod(c, 2)
        heads = [g * NH + j for j in range(NH)]
        idx_q = [d * H + h for h in heads for d in range(D)]
        idx_k = [((D - d) % D) * H + h for h in heads for d in range(D)]
        qTh, qTl = _split16(np.ascontiguousarray(q[b].T))
        kvTh, kvTl = _split16(np.ascontiguousarray(kv[b].T))
        wq_h, wq_l = _split16(Wq[:, idx_q] * np.float32(SCALE))
        wk_h, wk_l = _split16(Wk[:, idx_k])
        wv_c = np.zeros((E, DX), np.float16)
        for j, h in enumerate(heads):
            wv_c[:, j * (D + 1):j * (D + 1) + D] = \
                Wv[:, [d * H + h for d in range(D)]].astype(np.float16)
        in_maps.append({
            "qh": qTh, "ql": qTl, "kvh": kvTh, "kvl": kvTl,
            "wqh": wq_h, "wql": wq_l, "wkh": wk_h, "wkl": wk_l,
            "wv": wv_c,
            "wo": Wo[g * NH * D:(g + 1) * NH * D, :].astype(np.float16),
            "ident": np.eye(128, dtype=np.float32),
        })
    return in_maps


def kernel(**inputs):
    global LAST_EXEC_NS
    q = np.asarray(inputs["q"], dtype=np.float32)
    kv = np.asarray(inputs["kv"], dtype=np.float32)
    Wq = np.asarray(inputs["Wq"], dtype=np.float32)
    Wk = np.asarray(inputs["Wk"], dtype=np.float32)
    Wv = np.asarray(inputs["Wv"], dtype=np.float32)
    Wo = np.asarray(inputs["Wo"], dtype=np.float32)
    B = q.shape[0]

    nc = _get_program()
    in_maps = make_in_maps(q, kv, Wq, Wk, Wv, Wo)
    res = run_bass_kernel_spmd(nc, in_maps, list(range(N_CORES)), trace=TRACE)
    LAST_EXEC_NS = res.exec_time_ns

    out = np.empty((B, T, E), np.float32)
    for b in range(B):
        r0, r1 = res.results[2 * b], res.results[2 * b + 1]
        out[b] = (r0["out0"] + r0["out1"]) + (r1["out0"] + r1["out1"])
    return out
